# revision 1
# baseline (speedup 1.0000x reference)
"""Gemma3 sliding-window attention on 8 Trainium2 NeuronCores.

Sharding: core c handles batch b=c//4 and head-group g=c%4 (4 of 16 q heads,
2 of 8 kv heads). wq/wk/wv column-split, wo row-split; the 4 partial outputs
per batch are summed on host (no device collectives).

All device matmuls contract over the partition dim, so the host pre-transposes
hidden_states and weights. Q/K are produced transposed (d on partitions), V in
natural layout; scores are computed transposed ([k,q]) so softmax-normalisation
can be deferred (flash-style) and PV/output-projection need no transposes.
"""

import math
import numpy as np

import concourse.bacc as bacc
import concourse.mybir as mybir
import concourse.tile as tile
from concourse.bass_utils import run_bass_kernel_spmd

dt = mybir.dt
AFT = mybir.ActivationFunctionType

B, S, H = 2, 2048, 2048
NQ, NKV, D = 16, 8, 128          # global heads
NQC, NKVC = 4, 2                 # heads per core
WIN = 1024
EPS = 1e-6
THETA = 10000.0
NEG = -1.0e9
P = 128
SCP = 256                        # seq chunk: projections + attention (N>=256 keeps fp32r at 1cyc/row)
SCO = 512                        # seq chunk: output projection
NHT = H // P                     # 16 hidden tiles
NST = S // P                     # 16 seq tiles

_CACHE = {}
PHASES = 3
BUFS = {"hsp": 2, "tmp1": 3, "ps1": 3, "psv": 2, "ps1b": 2, "probs": 8, "ps2": 4, "psa": 1, "psd": 1, "psy": 2, "ysb": 6, "tabp": 2, "tmp2": 3}


def _build_nc():
    if "nc" in _CACHE:
        return _CACHE["nc"]
    nc = bacc.Bacc("TRN2", target_bir_lowering=False, debug=False, num_devices=8)
    f32, f32r = dt.float32, dt.float32r
    r = lambda ap: ap.bitcast(f32r)

    hsT = nc.dram_tensor("hsT", [H, S], f32r, kind="ExternalInput").ap()
    wqT = nc.dram_tensor("wqT", [H, NQC * D], f32r, kind="ExternalInput").ap()
    wkT = nc.dram_tensor("wkT", [H, NKVC * D], f32r, kind="ExternalInput").ap()
    wvT = nc.dram_tensor("wvT", [H, NKVC * D], f32r, kind="ExternalInput").ap()
    woT = nc.dram_tensor("woT", [NQC * D, H], f32r, kind="ExternalInput").ap()
    cosq = nc.dram_tensor("cosq", [D, S], f32, kind="ExternalInput").ap()
    sinq = nc.dram_tensor("sinq", [D, S], f32, kind="ExternalInput").ap()
    cosk = nc.dram_tensor("cosk", [D, S], f32, kind="ExternalInput").ap()
    sink = nc.dram_tensor("sink", [D, S], f32, kind="ExternalInput").ap()
    rqT = nc.dram_tensor("rqT", [D, D], f32r, kind="ExternalInput").ap()
    rkT = nc.dram_tensor("rkT", [D, D], f32r, kind="ExternalInput").ap()
    onesd = nc.dram_tensor("onesd", [P, P], f32r, kind="ExternalInput").ap()
    dmask = nc.dram_tensor("dmask", [P, P], f32, kind="ExternalInput").ap()
    emask = nc.dram_tensor("emask", [P, P], f32, kind="ExternalInput").ap()
    zmask = nc.dram_tensor("zmask", [P, P], f32, kind="ExternalInput").ap()
    yT = nc.dram_tensor("yT", [H, S], f32, kind="ExternalOutput").ap()

    nch = S // SCP               # 8 projection/attention chunks
    with tile.TileContext(nc) as tc:
        with (
            tc.tile_pool(name="const", bufs=1) as cpool,
            tc.tile_pool(name="qkv", bufs=1) as qkv,
        ):
            ones_sb = cpool.tile([P, P], f32r)
            nc.sync.dma_start(out=ones_sb[:], in_=onesd[:])
            dm_sb = cpool.tile([P, P], f32, tag="dm")
            em_sb = cpool.tile([P, P], f32, tag="em")
            zm_sb = cpool.tile([P, P], f32, tag="zm")
            eps_sb = cpool.tile([P, 1], f32, tag="eps")
            nc.vector.memset(eps_sb[:], EPS)
            rq_sb = cpool.tile([D, D], f32r, tag="rq")
            rk_sb = cpool.tile([D, D], f32r, tag="rk")
            nc.sync.dma_start(out=dm_sb[:], in_=dmask[:])
            nc.sync.dma_start(out=zm_sb[:], in_=zmask[:])
            nc.sync.dma_start(out=em_sb[:], in_=emask[:])
            nc.sync.dma_start(out=rq_sb[:], in_=rqT[:])
            nc.sync.dma_start(out=rk_sb[:], in_=rkT[:])

            qn_sb = qkv.tile([P, NQC, S], f32r, tag="qn")     # 4 MB
            kn_sb = qkv.tile([P, NKVC, S], f32r, tag="kn")    # 2 MB
            v_sb = qkv.tile([P, NST, NKVC * D], f32r, tag="v")  # 2 MB

            # ---------------- phase 1: QKV projections + RMSNorm + RoPE ----
            with (
                tc.tile_pool(name="w1", bufs=1) as w1,
                tc.tile_pool(name="hsp", bufs=BUFS["hsp"]) as hsp,
                tc.tile_pool(name="tabp", bufs=BUFS["tabp"]) as tabp,
                tc.tile_pool(name="tmp1", bufs=BUFS["tmp1"]) as tmp1,
                tc.tile_pool(name="ps1", bufs=BUFS["ps1"], space="PSUM") as ps1,
                tc.tile_pool(name="psv", bufs=BUFS["psv"], space="PSUM") as psv,
                tc.tile_pool(name="ps1b", bufs=BUFS["ps1b"], space="PSUM") as ps1b,
            ):
                wq_sb = w1.tile([P, NHT, NQC * D], f32r, tag="wq")
                wk_sb = w1.tile([P, NHT, NKVC * D], f32r, tag="wk")
                wv_sb = w1.tile([P, NHT, NKVC * D], f32r, tag="wv")
                for ht in range(NHT):
                    nc.sync.dma_start(out=wq_sb[:, ht, :], in_=wqT[ht * P:(ht + 1) * P, :])
                    nc.sync.dma_start(out=wk_sb[:, ht, :], in_=wkT[ht * P:(ht + 1) * P, :])
                    nc.sync.dma_start(out=wv_sb[:, ht, :], in_=wvT[ht * P:(ht + 1) * P, :])

                for sc in range(nch):
                    s0 = sc * SCP
                    hs_sb = hsp.tile([P, NHT, SCP], f32r, tag="hs")
                    for ht in range(NHT):
                        nc.sync.dma_start(out=hs_sb[:, ht, :], in_=hsT[ht * P:(ht + 1) * P, s0:s0 + SCP])
                    tabs = {}
                    for nm, ap in (("cosq", cosq), ("sinq", sinq), ("cosk", cosk), ("sink", sink)):
                        t = tabp.tile([D, SCP], f32, tag=nm)
                        nc.sync.dma_start(out=t[:], in_=ap[:, s0:s0 + SCP])
                        tabs[nm] = t

                    # q & k heads: transposed projection + norm + rope
                    for kind in ("q", "k"):
                        nheads = NQC if kind == "q" else NKVC
                        w_sb = wq_sb if kind == "q" else wk_sb
                        rot_sb = rq_sb if kind == "q" else rk_sb
                        cos_t = tabs["cosq" if kind == "q" else "cosk"]
                        sin_t = tabs["sinq" if kind == "q" else "sink"]
                        dst = qn_sb if kind == "q" else kn_sb
                        for m in range(nheads):
                            pp = ps1.tile([P, SCP], f32, tag="proj")
                            for ht in range(NHT):
                                nc.tensor.matmul(
                                    pp[:], r(w_sb[:, ht, m * D:(m + 1) * D]), r(hs_sb[:, ht, :]),
                                    start=(ht == 0), stop=(ht == NHT - 1))
                            sq = tmp1.tile([P, SCP], f32r, tag="sq")
                            nc.scalar.square(sq[:], pp[:])
                            vb = ps1b.tile([P, SCP], f32, tag="aux")
                            nc.tensor.matmul(vb[:], r(ones_sb[:]), r(sq[:]), start=True, stop=True)
                            sd = tmp1.tile([P, SCP], f32, tag="sd")
                            nc.scalar.activation(sd[:], vb[:], AFT.Sqrt, bias=eps_sb[:], scale=1.0 / D)
                            inv = tmp1.tile([P, SCP], f32, tag="inv")
                            nc.vector.reciprocal(inv[:], sd[:])
                            xn = tmp1.tile([P, SCP], f32r, tag="xn")
                            nc.vector.tensor_mul(xn[:], pp[:], inv[:])
                            rb = ps1b.tile([P, SCP], f32, tag="aux")
                            nc.tensor.matmul(rb[:], r(rot_sb[:]), r(xn[:]), start=True, stop=True)
                            tcos = tmp1.tile([P, SCP], f32, tag="tcos")
                            nc.vector.tensor_mul(tcos[:], xn[:], cos_t[:])
                            tsin = tmp1.tile([P, SCP], f32, tag="tsin")
                            nc.vector.tensor_mul(tsin[:], rb[:], sin_t[:])
                            nc.vector.tensor_add(dst[:, m, s0:s0 + SCP], tcos[:], tsin[:])

                    # v: natural layout
                    for ss in range(SCP // P):
                        st = sc * (SCP // P) + ss
                        pv = psv.tile([P, NKVC * D], f32, tag="vproj")
                        for ht in range(NHT):
                            nc.tensor.matmul(
                                pv[:], r(hs_sb[:, ht, ss * P:(ss + 1) * P]), r(wv_sb[:, ht, :]),
                                start=(ht == 0), stop=(ht == NHT - 1))
                        nc.vector.tensor_copy(v_sb[:, st, :], pv[:])

            # ---------------- phase 2+3: attention + output projection -----
            with (
                tc.tile_pool(name="attnp", bufs=1) as attnp,
                tc.tile_pool(name="wo", bufs=1) as wop,
                tc.tile_pool(name="probs", bufs=BUFS["probs"]) as probs,
                tc.tile_pool(name="tmp2", bufs=BUFS["tmp2"]) as tmp2,
                tc.tile_pool(name="ps2", bufs=BUFS["ps2"], space="PSUM") as ps2,
                tc.tile_pool(name="psa", bufs=BUFS["psa"], space="PSUM") as psa,
                tc.tile_pool(name="psd", bufs=BUFS["psd"], space="PSUM") as psd,
            ):
                attn_sb = attnp.tile([P, NQC, S], f32r, tag="attn")  # 4 MB
                wo_sb = wop.tile([P, NQC, H], f32r, tag="wo")
                for dto in range(NQC):
                    nc.sync.dma_start(out=wo_sb[:, dto, :], in_=woT[dto * P:(dto + 1) * P, :])

                nsub = SCP // P  # q subtiles per chunk (2)
                for h in range(NQC if PHASES >= 2 else 0):
                    kvh = h // 2
                    for qc in range(nch):
                        u0 = qc * nsub                       # first abs q tile
                        t0 = max(0, u0 - WIN // P)
                        t1 = u0 + nsub - 1                   # last k tile (causal)
                        ts = list(range(t0, t1 + 1))
                        a_ps = psa.tile([P, SCP], f32, tag="attn_ps")
                        d_ps = psd.tile([P, SCP], f32, tag="den_ps")
                        for ti, t in enumerate(ts):
                            s_ps = ps2.tile([P, SCP], f32, tag="scores")
                            nc.tensor.matmul(
                                s_ps[:], r(kn_sb[:, kvh, t * P:(t + 1) * P]),
                                r(qn_sb[:, h, qc * SCP:qc * SCP + SCP]),
                                start=True, stop=True)
                            p_sb = probs.tile([P, SCP], f32r, tag="p")
                            nc.scalar.activation(p_sb[:], s_ps[:], AFT.Exp)
                            for u in range(nsub):
                                dd = (u0 + u) - t
                                blk = p_sb[:, u * P:(u + 1) * P]
                                if dd == 0:
                                    nc.vector.tensor_mul(blk, blk, dm_sb[:])
                                elif dd == WIN // P:
                                    nc.vector.tensor_mul(blk, blk, em_sb[:])
                                elif dd < 0 or dd > WIN // P:
                                    nc.vector.tensor_mul(blk, blk, zm_sb[:])
                            first, last = ti == 0, ti == len(ts) - 1
                            nc.tensor.matmul(
                                a_ps[:], r(v_sb[:, t, kvh * D:(kvh + 1) * D]), r(p_sb[:]),
                                start=first, stop=last)
                            nc.tensor.matmul(
                                d_ps[:], r(ones_sb[:]), r(p_sb[:]), start=first, stop=last)
                        inv = tmp2.tile([P, SCP], f32, tag="dinv")
                        nc.vector.reciprocal(inv[:], d_ps[:])
                        nc.vector.tensor_mul(attn_sb[:, h, qc * SCP:qc * SCP + SCP], a_ps[:], inv[:])

                # output projection: yT[mo,:] = sum_h woT[h-block, mo-block].T @ attnT[h]
                with (
                    tc.tile_pool(name="psy", bufs=BUFS["psy"], space="PSUM") as psy,
                    tc.tile_pool(name="ysb", bufs=BUFS["ysb"]) as ysb,
                ):
                    for mo in range(NHT if PHASES >= 3 else 0):
                        for oc in range(S // SCO):
                            y_ps = psy.tile([P, SCO], f32, tag="y")
                            for h in range(NQC):
                                nc.tensor.matmul(
                                    y_ps[:], r(wo_sb[:, h, mo * P:(mo + 1) * P]),
                                    r(attn_sb[:, h, oc * SCO:oc * SCO + SCO]),
                                    start=(h == 0), stop=(h == NQC - 1))
                            y_sb = ysb.tile([P, SCO], f32, tag="ysb")
                            nc.vector.tensor_copy(y_sb[:], y_ps[:])
                            nc.sync.dma_start(
                                out=yT[mo * P:(mo + 1) * P, oc * SCO:oc * SCO + SCO],
                                in_=y_sb[:])

    nc.compile()
    _CACHE["nc"] = nc
    return nc


def _host_inputs(hidden_states, wq, wk, wv, wo, q_norm_weight, k_norm_weight):
    """Per-core input dicts (8 cores: c = 4*b + g)."""
    f = np.float32
    scale = 1.0 / math.sqrt(D)
    inv_freq = 1.0 / (THETA ** (np.arange(0, D, 2, dtype=np.float64) / D))
    t = np.arange(S, dtype=np.float64)
    freqs = np.outer(t, inv_freq)
    emb = np.concatenate([freqs, freqs], axis=-1)          # [S, D]
    cosT = np.cos(emb).T.astype(f)                         # [D, S]
    sinT = np.sin(emb).T.astype(f)
    qw = (1.0 + q_norm_weight).astype(f)
    kw = (1.0 + k_norm_weight).astype(f)

    R = np.zeros((D, D), f)
    hh = D // 2
    for i in range(hh):
        R[i, i + hh] = -1.0
        R[i + hh, i] = 1.0
    rqT = np.ascontiguousarray((R * qw[None, :]).T)
    rkT = np.ascontiguousarray((R * kw[None, :]).T)

    cosq = np.ascontiguousarray(cosT * qw[:, None] * scale)
    sinq = np.ascontiguousarray(sinT * scale)
    cosk = np.ascontiguousarray(cosT * kw[:, None])
    sink = np.ascontiguousarray(sinT)

    r = np.arange(P)[:, None]
    c = np.arange(P)[None, :]
    dmask = np.where(c >= r, 1.0, 0.0).astype(f)           # diag: q_col >= k_row
    emask = np.where(r > c, 1.0, 0.0).astype(f)            # edge: k_row > q_col

    hsT = [np.ascontiguousarray(hidden_states[b].T.astype(f)) for b in range(B)]
    in_maps = []
    for core in range(8):
        b, g = divmod(core, 4)
        in_maps.append({
            "hsT": hsT[b],
            "wqT": np.ascontiguousarray(wq[512 * g:512 * (g + 1), :].T.astype(f)),
            "wkT": np.ascontiguousarray(wk[256 * g:256 * (g + 1), :].T.astype(f)),
            "wvT": np.ascontiguousarray(wv[256 * g:256 * (g + 1), :].T.astype(f)),
            "woT": np.ascontiguousarray(wo[:, 512 * g:512 * (g + 1)].T.astype(f)),
            "cosq": cosq, "sinq": sinq, "cosk": cosk, "sink": sink,
            "rqT": rqT, "rkT": rkT, "onesd": np.ones((P, P), f),
            "dmask": dmask, "emask": emask, "zmask": np.zeros((P, P), f),
        })
    return in_maps


def _postprocess(results):
    out = np.empty((B, S, H), np.float32)
    for b in range(B):
        acc = results[4 * b]["yT"].astype(np.float32).copy()
        for g in range(1, 4):
            acc += results[4 * b + g]["yT"]
        out[b] = acc.T
    return out


def kernel(hidden_states, wq, wk, wv, wo, q_norm_weight, k_norm_weight):
    nc = _build_nc()
    in_maps = _host_inputs(hidden_states, wq, wk, wv, wo, q_norm_weight, k_norm_weight)
    res = run_bass_kernel_spmd(nc, in_maps, list(range(8)))
    return _postprocess(res.results)



# revision 6
# speedup vs baseline: 1.1617x; 1.1617x over previous
"""Gemma3 sliding-window attention on 8 Trainium2 NeuronCores.

Sharding: core c handles batch b=c//4 and head-group g=c%4 (4 of 16 q heads,
2 of 8 kv heads). wq/wk/wv column-split, wo row-split; the 4 partial outputs
per batch are summed on host (no device collectives).

v2: all matmul operands in bf16 (fp32 PSUM accumulation), single-instruction
batched DMA loads from host-prepacked layouts, per-q-tile(128) attention with
the softmax denominator computed as a 129th V column in a [q,d]-oriented PV
matmul, XBAR DMA transposes to return attn to [d,q] for the output
projection, and software-pipelined instruction issue so the PE never waits
on the RMSNorm/RoPE vector chains.
"""

import math
import numpy as np
import ml_dtypes

import concourse.bacc as bacc
import concourse.mybir as mybir
import concourse.tile as tile
from concourse.bass_utils import run_bass_kernel_spmd

dt = mybir.dt
AFT = mybir.ActivationFunctionType
BF = dt.bfloat16
F32 = dt.float32

B, S, H = 2, 2048, 2048
NQC, NKVC, D = 4, 2, 128          # per-core heads
WIN = 1024
EPS = 1e-6
THETA = 10000.0
P = 128
SCP = 512                          # phase-1 seq chunk
NCH = S // SCP                     # 4
NHT = H // P                       # 16
NST = S // P                       # 16
WT = WIN // P                      # 8 (window in tiles)
LAG = 2                            # attention PV pipeline depth (units)

_CACHE = {}


def _groups_for(t0, u0):
    """k-tile groups for one q tile: runs of <=4 tiles, diagonal tile alone
    last (so its [128,128] exp/mask stays separate)."""
    ts = list(range(t0, u0 + 1))
    if len(ts) == 1:
        return [ts]
    body, diag = ts[:-1], ts[-1:]
    gs = [body[i:i + 4] for i in range(0, len(body), 4)]
    gs.append(diag)
    return gs


def _build_nc():
    if "nc" in _CACHE:
        return _CACHE["nc"]
    nc = bacc.Bacc("TRN2", target_bir_lowering=False, debug=False, num_devices=8)

    hs_d = nc.dram_tensor("hs", [P, NHT, S], BF, kind="ExternalInput").ap()
    wq_d = nc.dram_tensor("wq", [P, NHT, NQC * D], BF, kind="ExternalInput").ap()
    wk_d = nc.dram_tensor("wk", [P, NHT, NKVC * D], BF, kind="ExternalInput").ap()
    wv_d = nc.dram_tensor("wv", [P, NHT, NKVC * D], BF, kind="ExternalInput").ap()
    wo_d = nc.dram_tensor("wo", [P, NQC, H], BF, kind="ExternalInput").ap()
    tabs_d = nc.dram_tensor("tabs", [P, 4, S], BF, kind="ExternalInput").ap()
    rots_d = nc.dram_tensor("rots", [P, 2, P], BF, kind="ExternalInput").ap()
    msk_d = nc.dram_tensor("msk", [P, 2, P], BF, kind="ExternalInput").ap()
    y_d = nc.dram_tensor("y", [P, NHT, S], BF, kind="ExternalOutput").ap()

    with nc.allow_low_precision(reason="bf16 kernel; rel-err budget 2e-2"), \
         tile.TileContext(nc) as tc:
        with (
            tc.tile_pool(name="const", bufs=1) as cpool,
            tc.tile_pool(name="qkv", bufs=1) as qkv,
            tc.tile_pool(name="wts", bufs=1) as wts,
        ):
            msk_sb = cpool.tile([P, 2, P], BF, tag="msk")
            rots_sb = cpool.tile([P, 2, P], BF, tag="rots")
            ones_sb = cpool.tile([P, P], BF, tag="ones")
            eps_sb = cpool.tile([P, 1], F32, tag="eps")
            nc.sync.dma_start(out=msk_sb[:], in_=msk_d[:])
            nc.sync.dma_start(out=rots_sb[:], in_=rots_d[:])
            nc.vector.memset(ones_sb[:], 1.0)
            nc.vector.memset(eps_sb[:], EPS)
            dm_sb = msk_sb[:, 0, :]
            em_sb = msk_sb[:, 1, :]

            # weight loads: wv first (v-projection is the startup filler work),
            # then wk (k heads run before q heads), wq, wo last-needed.
            wv_sb = wts.tile([P, NHT, NKVC * D], BF, tag="wv")
            wk_sb = wts.tile([P, NHT, NKVC * D], BF, tag="wk")
            wq_sb = wts.tile([P, NHT, NQC * D], BF, tag="wq")
            wo_sb = wts.tile([P, NQC, H], BF, tag="wo")
            nc.sync.dma_start(out=wv_sb[:], in_=wv_d[:])

            qn_sb = qkv.tile([P, NQC, S], BF, tag="qn")
            kn_sb = qkv.tile([P, NKVC, S], BF, tag="kn")
            v_sb = qkv.tile([P, NST, NKVC, D + 1], BF, tag="v")
            nc.vector.memset(v_sb[:, :, :, D:D + 1], 1.0)

            # ---------------- phase 1: QKV projections + RMSNorm + RoPE ----
            # per (chunk, head): PE proj chain -> Act copy -> DVE square /
            # rope muls; the sum-of-squares and rotation matmuls for head m
            # are issued after head m+1's projection chain so PE never waits.
            with (
                tc.tile_pool(name="hsp", bufs=2) as hsp,
                tc.tile_pool(name="tabp", bufs=2) as tabp,
                tc.tile_pool(name="cpp", bufs=2) as cpp,
                tc.tile_pool(name="t1", bufs=2) as t1p,
                tc.tile_pool(name="t2", bufs=2) as t2p,
                tc.tile_pool(name="t3", bufs=2) as t3p,
                tc.tile_pool(name="t4", bufs=2) as t4p,
                tc.tile_pool(name="t5", bufs=2) as t5p,
                tc.tile_pool(name="t6", bufs=2) as t6p,
                tc.tile_pool(name="pp", bufs=2, space="PSUM") as ppp,
                tc.tile_pool(name="prb", bufs=2, space="PSUM") as prbp,
                tc.tile_pool(name="pvb", bufs=2, space="PSUM") as pvbp,
                tc.tile_pool(name="psv", bufs=2, space="PSUM") as psvp,
            ):
                # heads order: k0, k1, q0..q3 (wk arrives before wq)
                HEADS = [("k", 0), ("k", 1), ("q", 0), ("q", 1), ("q", 2), ("q", 3)]
                pend = None  # deferred norm/rope finish of the previous head

                def finish(st):
                    kind, m, pp, cp, u_t, s0, tab_t = st
                    sq = t1p.tile([P, SCP], BF, tag="sq")
                    nc.vector.tensor_mul(sq[:], cp[:], cp[:])
                    rb = prbp.tile([P, SCP], F32, tag="rb")
                    rot = rots_sb[:, 0, :] if kind == "q" else rots_sb[:, 1, :]
                    nc.tensor.matmul(rb[:], rot, cp[:], start=True, stop=True)
                    vb = pvbp.tile([P, SCP], F32, tag="vb")
                    nc.tensor.matmul(vb[:], ones_sb[:], sq[:], start=True, stop=True)
                    sd = t2p.tile([P, SCP], F32, tag="sd")
                    nc.scalar.activation(sd[:], vb[:], AFT.Sqrt, bias=eps_sb[:],
                                         scale=1.0 / D)
                    inv = t3p.tile([P, SCP], BF, tag="inv")
                    nc.vector.reciprocal(inv[:], sd[:])
                    # tsin: rb (PSUM) is ready late
                    tsin = t4p.tile([P, SCP], BF, tag="tsin")
                    sin_t = tab_t[:, 1 if kind == "q" else 3, :]
                    nc.vector.tensor_mul(tsin[:], rb[:], sin_t)
                    nc.vector.tensor_add(u_t[:], u_t[:], tsin[:])
                    dst = qn_sb if kind == "q" else kn_sb
                    nc.vector.tensor_mul(dst[:, m, s0:s0 + SCP], u_t[:], inv[:])

                for sc in range(NCH):
                    s0 = sc * SCP
                    hs_sb = hsp.tile([P, NHT, SCP], BF, tag="hs")
                    nc.sync.dma_start(out=hs_sb[:], in_=hs_d[:, :, s0:s0 + SCP])
                    if sc == 0:
                        nc.sync.dma_start(out=wk_sb[:], in_=wk_d[:])
                        nc.sync.dma_start(out=wq_sb[:], in_=wq_d[:])
                    tab_sb = tabp.tile([P, 4, SCP], BF, tag="tab")
                    nc.sync.dma_start(out=tab_sb[:], in_=tabs_d[:, :, s0:s0 + SCP])
                    if sc == 0:
                        nc.sync.dma_start(out=wo_sb[:], in_=wo_d[:])

                    # v projection: natural [seq, d] layout + filler work
                    for ss in range(SCP // P):
                        st_g = sc * (SCP // P) + ss
                        pv = psvp.tile([P, NKVC * D], F32, tag="pv")
                        for ht in range(NHT):
                            nc.tensor.matmul(
                                pv[:], hs_sb[:, ht, ss * P:(ss + 1) * P],
                                wv_sb[:, ht, :],
                                start=(ht == 0), stop=(ht == NHT - 1))
                        nc.vector.tensor_copy(v_sb[:, st_g, :, 0:D], pv[:])

                    for kind, m in HEADS:
                        w_sb = wq_sb if kind == "q" else wk_sb
                        pp = ppp.tile([P, SCP], F32, tag="pp")
                        for ht in range(NHT):
                            nc.tensor.matmul(
                                pp[:], w_sb[:, ht, m * D:(m + 1) * D],
                                hs_sb[:, ht, :],
                                start=(ht == 0), stop=(ht == NHT - 1))
                        cp = cpp.tile([P, SCP], BF, tag="cp")
                        nc.scalar.copy(cp[:], pp[:])
                        u_t = t5p.tile([P, SCP], BF, tag="u")
                        cos_t = tab_sb[:, 0 if kind == "q" else 2, :]
                        nc.vector.tensor_mul(u_t[:], cp[:], cos_t)
                        if pend is not None:
                            finish(pend)
                        pend = (kind, m, pp, cp, u_t, s0, tab_sb)
                finish(pend)

            # ---------------- phase 2+3: attention + output projection -----
            with (
                tc.tile_pool(name="pb", bufs=4) as pbp,
                tc.tile_pool(name="invp", bufs=4) as invp,
                tc.tile_pool(name="aq", bufs=2) as aqp,
                tc.tile_pool(name="aT", bufs=2) as aTp,
                tc.tile_pool(name="ysb", bufs=2) as ysp,
                tc.tile_pool(name="psc", bufs=4, space="PSUM") as pscp,
                tc.tile_pool(name="pa", bufs=2, space="PSUM") as pap,
                tc.tile_pool(name="psy", bufs=2, space="PSUM") as psyp,
            ):
                queue = []
                slab_tiles = {}

                def emit_scores(h, u0):
                    kvh = h // 2
                    t0 = max(0, u0 - WT)
                    gs = _groups_for(t0, u0)
                    p_t = pbp.tile([P, (WT + 1) * P], BF, tag="p")
                    for g in gs:
                        sc_t = pscp.tile([P, 4 * P], F32, tag="sc")
                        for i, t in enumerate(g):
                            nc.tensor.matmul(
                                sc_t[:, i * P:(i + 1) * P],
                                kn_sb[:, kvh, t * P:(t + 1) * P],
                                qn_sb[:, h, u0 * P:(u0 + 1) * P],
                                start=True, stop=True)
                        off = (g[0] - t0) * P
                        n = len(g) * P
                        nc.scalar.activation(p_t[:, off:off + n], sc_t[:, 0:n],
                                             AFT.Exp)
                        for t in g:
                            if t == u0:
                                blk = p_t[:, (t - t0) * P:(t - t0 + 1) * P]
                                nc.vector.tensor_mul(blk, blk, dm_sb)
                            elif u0 >= WT and t == u0 - WT:
                                blk = p_t[:, (t - t0) * P:(t - t0 + 1) * P]
                                nc.vector.tensor_mul(blk, blk, em_sb)
                    return (h, u0, t0, p_t)

                def emit_pv(st):
                    h, u0, t0, p_t = st
                    kvh = h // 2
                    ts = list(range(t0, u0 + 1))
                    a_t = pap.tile([P, D + 1], F32, tag="a")
                    for i, t in enumerate(ts):
                        nc.tensor.matmul(
                            a_t[:], p_t[:, i * P:(i + 1) * P],
                            v_sb[:, t, kvh, :],
                            start=(i == 0), stop=(i == len(ts) - 1))
                    inv = invp.tile([P, 1], F32, tag="inv")
                    nc.vector.reciprocal(inv[:], a_t[:, D:D + 1])
                    slab = slab_tiles[u0 // 4]
                    nc.scalar.activation(slab[:, h, u0 % 4, :], a_t[:, 0:D],
                                         AFT.Copy, bias=0.0, scale=inv[:])

                def emit_transposes(s4):
                    aT = aTp.tile([P, NQC, 4, P], BF, tag="aT", name="aT")
                    slab = slab_tiles[s4]
                    for h in range(NQC):
                        nc.sync.dma_start_transpose(out=aT[:, h, :, :],
                                                    in_=slab[:, h, :, :])
                    slab_tiles[("T", s4)] = aT

                def emit_outproj(s4):
                    aT = slab_tiles[("T", s4)]
                    for mog in range(4):
                        y_t = ysp.tile([P, 4, SCP], BF, tag="y")
                        for mo4 in range(4):
                            mo = mog * 4 + mo4
                            yp = psyp.tile([P, SCP], F32, tag="yp")
                            for h in range(NQC):
                                nc.tensor.matmul(
                                    yp[:], wo_sb[:, h, mo * P:(mo + 1) * P],
                                    aT[:, h, :, :],
                                    start=(h == 0), stop=(h == NQC - 1))
                            nc.vector.tensor_copy(y_t[:, mo4, :], yp[:])
                        nc.sync.dma_start(
                            out=y_d[:, mog * 4:(mog + 1) * 4,
                                    s4 * SCP:(s4 + 1) * SCP],
                            in_=y_t[:])

                for u0 in range(NST):
                    if u0 % 4 == 0:
                        slab_tiles[u0 // 4] = aqp.tile([P, NQC, 4, P], BF,
                                                       tag="aq", name="aq")
                    if u0 % 4 == 1 and u0 >= 5:
                        emit_transposes((u0 - 5) // 4)
                    for h in range(NQC):
                        queue.append(emit_scores(h, u0))
                        if len(queue) > LAG:
                            emit_pv(queue.pop(0))
                    if u0 % 4 == 3 and u0 >= 7:
                        emit_outproj((u0 - 7) // 4)
                while queue:
                    emit_pv(queue.pop(0))
                emit_transposes(3)
                emit_outproj(3)

    nc.compile()
    _CACHE["nc"] = nc
    return nc


def _host_inputs(hidden_states, wq, wk, wv, wo, q_norm_weight, k_norm_weight):
    """Per-core input dicts (8 cores: c = 4*b + g)."""
    bf = ml_dtypes.bfloat16
    f = np.float32
    scale = 1.0 / math.sqrt(D)
    inv_freq = 1.0 / (THETA ** (np.arange(0, D, 2, dtype=np.float64) / D))
    t = np.arange(S, dtype=np.float64)
    freqs = np.outer(t, inv_freq)
    emb = np.concatenate([freqs, freqs], axis=-1)          # [S, D]
    cosT = np.cos(emb).T.astype(np.float64)                # [D, S]
    sinT = np.sin(emb).T.astype(np.float64)
    qw = (1.0 + q_norm_weight).astype(np.float64)
    kw = (1.0 + k_norm_weight).astype(np.float64)

    # rotate-half matrices with norm weights folded (lhsT layout, like
    # baseline): rb = rots.T @ x = (R * w) @ x
    R = np.zeros((D, D), np.float64)
    hh = D // 2
    for i in range(hh):
        R[i, i + hh] = -1.0
        R[i + hh, i] = 1.0
    rqT = np.ascontiguousarray((R * qw[None, :]).T)
    rkT = np.ascontiguousarray((R * kw[None, :]).T)
    rots = np.stack([rqT, rkT], axis=1).astype(bf)         # [D, 2, D]

    tabs = np.stack([
        cosT * qw[:, None] * scale,
        sinT * scale,
        cosT * kw[:, None],
        sinT,
    ], axis=1).astype(bf)                                  # [D, 4, S]

    r = np.arange(P)[:, None]
    c = np.arange(P)[None, :]
    dmask = np.where(c >= r, 1.0, 0.0)                     # [k, q]: q >= k
    emask = np.where(c < r, 1.0, 0.0)                      # [k, q]: q < k
    msk = np.stack([dmask, emask], axis=1).astype(bf)      # [128, 2, 128]

    def pack_w(w_slice):
        # [O, H] -> lhsT [H, O] -> [128, NHT, O]
        wT = w_slice.T.astype(np.float64)
        O = wT.shape[1]
        return np.ascontiguousarray(
            wT.reshape(NHT, P, O).transpose(1, 0, 2)).astype(bf)

    hs_packed = []
    for b in range(B):
        hsT = hidden_states[b].T.astype(np.float64)        # [H, S]
        hs_packed.append(np.ascontiguousarray(
            hsT.reshape(NHT, P, S).transpose(1, 0, 2)).astype(bf))

    in_maps = []
    for core in range(8):
        b, g = divmod(core, 4)
        woT = wo[:, 512 * g:512 * (g + 1)].T.astype(np.float64)  # [512, H]
        wo_r = np.ascontiguousarray(
            woT.reshape(NQC, P, H).transpose(1, 0, 2)).astype(bf)
        in_maps.append({
            "hs": hs_packed[b],
            "wq": pack_w(wq[512 * g:512 * (g + 1), :]),
            "wk": pack_w(wk[256 * g:256 * (g + 1), :]),
            "wv": pack_w(wv[256 * g:256 * (g + 1), :]),
            "wo": wo_r,
            "tabs": tabs, "rots": rots, "msk": msk,
        })
    return in_maps


def _postprocess(results):
    out = np.empty((B, S, H), np.float32)
    for b in range(B):
        acc = np.zeros((H, S), np.float32)
        for g in range(4):
            y_r = results[4 * b + g]["y"].astype(np.float32)  # [128, 16, S]
            acc += y_r.transpose(1, 0, 2).reshape(H, S)
        out[b] = acc.T
    return out


def kernel(hidden_states, wq, wk, wv, wo, q_norm_weight, k_norm_weight):
    nc = _build_nc()
    in_maps = _host_inputs(hidden_states, wq, wk, wv, wo,
                           q_norm_weight, k_norm_weight)
    res = run_bass_kernel_spmd(nc, in_maps, list(range(8)))
    return _postprocess(res.results)


# revision 27
# speedup vs baseline: 1.2942x; 1.1141x over previous
"""Gemma3 sliding-window attention on 8 Trainium2 NeuronCores.

Sharding: core c handles batch b=c//4 and head-group g=c%4 (4 of 16 q heads,
2 of 8 kv heads). wq/wk/wv column-split, wo row-split; the 4 partial outputs
per batch are summed on host (no device collectives).

v2: all matmul operands in bf16 (fp32 PSUM accumulation), single-instruction
batched DMA loads from host-prepacked layouts, per-q-tile(128) attention with
the softmax denominator computed as a 129th V column in a [q,d]-oriented PV
matmul, XBAR DMA transposes to return attn to [d,q] for the output
projection, and software-pipelined instruction issue so the PE never waits
on the RMSNorm/RoPE vector chains.
"""

import math
import numpy as np
import ml_dtypes

import concourse.bacc as bacc
import concourse.mybir as mybir
import concourse.tile as tile
from concourse.bass_utils import run_bass_kernel_spmd

dt = mybir.dt
AFT = mybir.ActivationFunctionType
BF = dt.bfloat16
F32 = dt.float32

B, S, H = 2, 2048, 2048
NQC, NKVC, D = 4, 2, 128          # per-core heads
WIN = 1024
EPS = 1e-6
THETA = 10000.0
P = 128
SCP = 512                          # phase-1 seq chunk
NCH = S // SCP                     # 4
NHT = H // P                       # 16
NST = S // P                       # 16
WT = WIN // P                      # 8 (window in tiles)
LAG = 4                            # attention PV pipeline depth (units)

_CACHE = {}


def _groups_for(t0, u0):
    """k-tile groups for one q tile: runs of <=4 tiles, diagonal tile alone
    last (so its [128,128] exp/mask stays separate)."""
    ts = list(range(t0, u0 + 1))
    if len(ts) == 1:
        return [ts]
    body, diag = ts[:-1], ts[-1:]
    gs = [body[i:i + 4] for i in range(0, len(body), 4)]
    gs.append(diag)
    return gs


def _build_nc():
    if "nc" in _CACHE:
        return _CACHE["nc"]
    nc = bacc.Bacc("TRN2", target_bir_lowering=False, debug=False, num_devices=8)

    hs_d = nc.dram_tensor("hs", [P, NCH, 4, NHT, P], BF, kind="ExternalInput").ap()
    wq_d = nc.dram_tensor("wq", [P, NHT, NQC * D], BF, kind="ExternalInput").ap()
    wk_d = nc.dram_tensor("wk", [P, NHT, NKVC * D], BF, kind="ExternalInput").ap()
    wv_d = nc.dram_tensor("wv", [P, NHT, NKVC * D], BF, kind="ExternalInput").ap()
    wo_d = nc.dram_tensor("wo", [P, NQC, H], BF, kind="ExternalInput").ap()
    tabs_d = nc.dram_tensor("tabs", [P, 4, S], BF, kind="ExternalInput").ap()
    rots_d = nc.dram_tensor("rots", [P, 2, P], BF, kind="ExternalInput").ap()
    msk_d = nc.dram_tensor("msk", [P, 2, P], BF, kind="ExternalInput").ap()
    y_d = nc.dram_tensor("y", [P, NHT, S], BF, kind="ExternalOutput").ap()

    with nc.allow_low_precision(reason="bf16 kernel; rel-err budget 2e-2"), \
         tile.TileContext(nc) as tc:
        with (
            tc.tile_pool(name="const", bufs=1) as cpool,
            tc.tile_pool(name="qkv", bufs=1) as qkv,
            tc.tile_pool(name="wts", bufs=1) as wts,
        ):
            msk_sb = cpool.tile([P, 2, P], BF, tag="msk")
            rots_sb = cpool.tile([P, 2, P], BF, tag="rots")
            ones_sb = cpool.tile([P, P], BF, tag="ones")
            eps_sb = cpool.tile([P, 1], F32, tag="eps")
            nc.vector.memset(ones_sb[:], 1.0)
            nc.vector.memset(eps_sb[:], EPS)
            dm_sb = msk_sb[:, 0, :]
            em_sb = msk_sb[:, 1, :]

            # weight loads: wv first (v-projection is the startup filler work),
            # then wk (k heads run before q heads), wq, wo last-needed.
            wv_sb = wts.tile([P, NHT, NKVC * D], BF, tag="wv")
            wk_sb = wts.tile([P, NHT, NKVC * D], BF, tag="wk")
            wq_sb = wts.tile([P, NHT, NQC * D], BF, tag="wq")
            wo_sb = wts.tile([P, NQC, H], BF, tag="wo")

            qn_sb = qkv.tile([P, NQC, S], BF, tag="qn")
            kn_sb = qkv.tile([P, NKVC, S], BF, tag="kn")
            v_sb = qkv.tile([P, NST, NKVC, D + 1], BF, tag="v")
            nc.vector.memset(v_sb[:, :, :, D:D + 1], 1.0)

            # ---------------- phase 1: QKV projections + RMSNorm + RoPE ----
            # per (chunk, head): PE proj chain -> Act copy -> DVE square /
            # rope muls; the sum-of-squares and rotation matmuls for head m
            # are issued after head m+1's projection chain so PE never waits.
            with (
                tc.tile_pool(name="hsp", bufs=2) as hsp,
                tc.tile_pool(name="tabp", bufs=2) as tabp,
                tc.tile_pool(name="cpp", bufs=2) as cpp,
                tc.tile_pool(name="t1", bufs=2) as t1p,
                tc.tile_pool(name="t2", bufs=2) as t2p,
                tc.tile_pool(name="t3", bufs=2) as t3p,
                tc.tile_pool(name="t4", bufs=2) as t4p,
                tc.tile_pool(name="t5", bufs=2) as t5p,
                tc.tile_pool(name="t6", bufs=2) as t6p,
                tc.tile_pool(name="pp", bufs=2, space="PSUM") as ppp,
                tc.tile_pool(name="prb", bufs=2, space="PSUM") as prbp,
                tc.tile_pool(name="pvb", bufs=2, space="PSUM") as pvbp,
                tc.tile_pool(name="psv", bufs=2, space="PSUM") as psvp,
            ):
                # heads order: k0, k1, q0..q3 (wk arrives before wq)
                HEADS = [("k", 0), ("k", 1), ("q", 0), ("q", 1), ("q", 2), ("q", 3)]
                pend = None  # deferred norm/rope finish of the previous head

                def finish(st):
                    kind, m, pp, cp, u_t, s0, tab_t = st
                    sq = t1p.tile([P, SCP], BF, tag="sq")
                    nc.vector.tensor_mul(sq[:], cp[:], cp[:])
                    rb = prbp.tile([P, SCP], F32, tag="rb")
                    rot = rots_sb[:, 0, :] if kind == "q" else rots_sb[:, 1, :]
                    nc.tensor.matmul(rb[:], rot, cp[:], start=True, stop=True)
                    vb = pvbp.tile([P, SCP], F32, tag="vb")
                    nc.tensor.matmul(vb[:], ones_sb[:], sq[:], start=True, stop=True)
                    sd = t2p.tile([P, SCP], F32, tag="sd")
                    nc.scalar.activation(sd[:], vb[:], AFT.Sqrt, bias=eps_sb[:],
                                         scale=1.0 / D)
                    inv = t3p.tile([P, SCP], BF, tag="inv")
                    nc.vector.reciprocal(inv[:], sd[:])
                    # tsin: rb (PSUM) is ready late
                    tsin = t4p.tile([P, SCP], BF, tag="tsin")
                    sin_t = tab_t[:, 1 if kind == "q" else 3, :]
                    nc.vector.tensor_mul(tsin[:], rb[:], sin_t)
                    nc.vector.tensor_add(u_t[:], u_t[:], tsin[:])
                    dst = qn_sb if kind == "q" else kn_sb
                    nc.vector.tensor_mul(dst[:, m, s0:s0 + SCP], u_t[:], inv[:])

                for sc in range(NCH):
                    s0 = sc * SCP
                    hs_sb = hsp.tile([P, 4, NHT, P], BF, tag="hs")
                    if sc == 0:
                        # startup-critical order: interleave hs quarters with
                        # wv halves so the v projection's first matmuls start
                        # ASAP, then wk/wq for the k/q chains.
                        hf = NHT // 2
                        nc.sync.dma_start(out=hs_sb[:, 0, :, :],
                                          in_=hs_d[:, 0, 0, :, :])
                        nc.sync.dma_start(out=wv_sb[:, 0:hf, :],
                                          in_=wv_d[:, 0:hf, :])
                        nc.sync.dma_start(out=hs_sb[:, 1, :, :],
                                          in_=hs_d[:, 0, 1, :, :])
                        nc.sync.dma_start(out=wv_sb[:, hf:NHT, :],
                                          in_=wv_d[:, hf:NHT, :])
                        nc.sync.dma_start(out=hs_sb[:, 2, :, :],
                                          in_=hs_d[:, 0, 2, :, :])
                        nc.sync.dma_start(out=hs_sb[:, 3, :, :],
                                          in_=hs_d[:, 0, 3, :, :])
                        nc.sync.dma_start(out=wk_sb[:], in_=wk_d[:])
                        nc.sync.dma_start(out=msk_sb[:], in_=msk_d[:])
                        nc.sync.dma_start(out=rots_sb[:], in_=rots_d[:])
                        nc.sync.dma_start(out=wq_sb[:], in_=wq_d[:])
                    else:
                        nc.sync.dma_start(out=hs_sb[:], in_=hs_d[:, sc, :, :, :])
                    tab_sb = tabp.tile([P, 4, SCP], BF, tag="tab")
                    nc.sync.dma_start(out=tab_sb[:], in_=tabs_d[:, :, s0:s0 + SCP])
                    if sc == 0:
                        nc.sync.dma_start(out=wo_sb[:], in_=wo_d[:])

                    # v projection: natural [seq, d] layout + filler work
                    for ss in range(SCP // P):
                        st_g = sc * (SCP // P) + ss
                        pv = psvp.tile([P, NKVC * D], F32, tag="pv")
                        for ht in range(NHT):
                            nc.tensor.matmul(
                                pv[:], hs_sb[:, ss, ht, :],
                                wv_sb[:, ht, :],
                                start=(ht == 0), stop=(ht == NHT - 1))
                        nc.vector.tensor_copy(v_sb[:, st_g, :, 0:D], pv[:])

                    for kind, m in HEADS:
                        w_sb = wq_sb if kind == "q" else wk_sb
                        pp = ppp.tile([P, SCP], F32, tag="pp")
                        for ht in range(NHT):
                            nc.tensor.matmul(
                                pp[:], w_sb[:, ht, m * D:(m + 1) * D],
                                hs_sb[:, :, ht, :],
                                start=(ht == 0), stop=(ht == NHT - 1))
                        cp = cpp.tile([P, SCP], BF, tag="cp")
                        nc.scalar.copy(cp[:], pp[:])
                        u_t = t5p.tile([P, SCP], BF, tag="u")
                        cos_t = tab_sb[:, 0 if kind == "q" else 2, :]
                        nc.vector.tensor_mul(u_t[:], cp[:], cos_t)
                        if pend is not None:
                            finish(pend)
                        pend = (kind, m, pp, cp, u_t, s0, tab_sb)
                finish(pend)

            # ---------------- phase 2+3: attention + output projection -----
            with (
                tc.tile_pool(name="pb", bufs=LAG + 2) as pbp,
                tc.tile_pool(name="invp", bufs=4) as invp,
                tc.tile_pool(name="aq", bufs=2) as aqp,
                tc.tile_pool(name="aT", bufs=2) as aTp,
                tc.tile_pool(name="ysb", bufs=2) as ysp,
                tc.tile_pool(name="psc", bufs=4, space="PSUM") as pscp,
                tc.tile_pool(name="pa", bufs=2, space="PSUM") as pap,
                tc.tile_pool(name="psy", bufs=2, space="PSUM") as psyp,
            ):
                queue = []
                slab_tiles = {}

                def emit_scores(h, u0):
                    kvh = h // 2
                    t0 = max(0, u0 - WT)
                    gs = _groups_for(t0, u0)
                    p_t = pbp.tile([P, (WT + 1) * P], BF, tag="p")
                    for g in gs:
                        sc_t = pscp.tile([P, 4 * P], F32, tag="sc")
                        for i, t in enumerate(g):
                            nc.tensor.matmul(
                                sc_t[:, i * P:(i + 1) * P],
                                kn_sb[:, kvh, t * P:(t + 1) * P],
                                qn_sb[:, h, u0 * P:(u0 + 1) * P],
                                start=True, stop=True)
                        off = (g[0] - t0) * P
                        n = len(g) * P
                        nc.scalar.activation(p_t[:, off:off + n], sc_t[:, 0:n],
                                             AFT.Exp)
                        for t in g:
                            # masks on the otherwise-idle GPSIMD engine: DVE
                            # runs ~90% busy in this phase and its queue
                            # latency was stalling the PV chains
                            if t == u0:
                                blk = p_t[:, (t - t0) * P:(t - t0 + 1) * P]
                                nc.vector.tensor_mul(blk, blk, dm_sb)
                            elif u0 >= WT and t == u0 - WT:
                                blk = p_t[:, (t - t0) * P:(t - t0 + 1) * P]
                                nc.vector.tensor_mul(blk, blk, em_sb)
                    return (h, u0, t0, p_t)

                def emit_pv(st):
                    h, u0, t0, p_t = st
                    kvh = h // 2
                    ts = list(range(t0, u0 + 1))
                    a_t = pap.tile([P, D + 1], F32, tag="a")
                    for i, t in enumerate(ts):
                        nc.tensor.matmul(
                            a_t[:], p_t[:, i * P:(i + 1) * P],
                            v_sb[:, t, kvh, :],
                            start=(i == 0), stop=(i == len(ts) - 1))
                    inv = invp.tile([P, 1], F32, tag="inv")
                    nc.vector.reciprocal(inv[:], a_t[:, D:D + 1])
                    slab = slab_tiles[u0 // 4]
                    nc.vector.tensor_scalar_mul(slab[:, h, u0 % 4, :],
                                                a_t[:, 0:D], inv[:])

                def emit_transpose(s4, h):
                    # Act HWDGE queue: avoids head-of-line blocking behind the
                    # SP queue's y-out DMAs (which wait on DVE copies). Issued
                    # one per unit so the Act SEQ time (~0.7us per DMA) does
                    # not delay exp dispatch in a lump.
                    if h == 0:
                        aT = aTp.tile([P, NQC, 4, P], BF, tag="aT", name="aT")
                        slab_tiles[("T", s4)] = aT
                    aT = slab_tiles[("T", s4)]
                    slab = slab_tiles[s4]
                    nc.scalar.dma_start_transpose(out=aT[:, h, :, :],
                                                  in_=slab[:, h, :, :])

                def emit_outproj(s4):
                    aT = slab_tiles[("T", s4)]
                    for mog in range(4):
                        y_t = ysp.tile([P, 4, SCP], BF, tag="y")
                        for mo4 in range(4):
                            mo = mog * 4 + mo4
                            yp = psyp.tile([P, SCP], F32, tag="yp")
                            for h in range(NQC):
                                nc.tensor.matmul(
                                    yp[:], wo_sb[:, h, mo * P:(mo + 1) * P],
                                    aT[:, h, :, :],
                                    start=(h == 0), stop=(h == NQC - 1))
                            nc.vector.tensor_copy(y_t[:, mo4, :], yp[:])
                        nc.sync.dma_start(
                            out=y_d[:, mog * 4:(mog + 1) * 4,
                                    s4 * SCP:(s4 + 1) * SCP],
                            in_=y_t[:])

                for u0 in range(NST):
                    if u0 % 4 == 0:
                        slab_tiles[u0 // 4] = aqp.tile([P, NQC, 4, P], BF,
                                                       tag="aq", name="aq")
                    if u0 % 4 == 3 and u0 >= 7:
                        emit_outproj((u0 - 7) // 4)
                    for h in range(NQC):
                        queue.append(emit_scores(h, u0))
                        if len(queue) > LAG:
                            emit_pv(queue.pop(0))
                        if u0 == NST - 1 and queue:
                            emit_pv(queue.pop(0))  # drain early for the tail
                        if u0 % 4 == 1 and u0 >= 5:
                            emit_transpose((u0 - 5) // 4, h)
                while queue:
                    emit_pv(queue.pop(0))
                # last slab: split transposes across both HWDGE queues to
                # halve the serial latency in the tail
                aT = aTp.tile([P, NQC, 4, P], BF, tag="aT", name="aT")
                slab_tiles[("T", 3)] = aT
                slab = slab_tiles[3]
                for h in range(NQC):
                    eng = nc.scalar if h % 2 == 0 else nc.sync
                    eng.dma_start_transpose(out=aT[:, h, :, :],
                                            in_=slab[:, h, :, :])
                emit_outproj(3)

    nc.compile()
    _CACHE["nc"] = nc
    return nc


def _host_inputs(hidden_states, wq, wk, wv, wo, q_norm_weight, k_norm_weight):
    """Per-core input dicts (8 cores: c = 4*b + g)."""
    bf = ml_dtypes.bfloat16
    f = np.float32
    scale = 1.0 / math.sqrt(D)
    inv_freq = 1.0 / (THETA ** (np.arange(0, D, 2, dtype=np.float64) / D))
    t = np.arange(S, dtype=np.float64)
    freqs = np.outer(t, inv_freq)
    emb = np.concatenate([freqs, freqs], axis=-1)          # [S, D]
    cosT = np.cos(emb).T.astype(np.float64)                # [D, S]
    sinT = np.sin(emb).T.astype(np.float64)
    qw = (1.0 + q_norm_weight).astype(np.float64)
    kw = (1.0 + k_norm_weight).astype(np.float64)

    # rotate-half matrices with norm weights folded (lhsT layout, like
    # baseline): rb = rots.T @ x = (R * w) @ x
    R = np.zeros((D, D), np.float64)
    hh = D // 2
    for i in range(hh):
        R[i, i + hh] = -1.0
        R[i + hh, i] = 1.0
    rqT = np.ascontiguousarray((R * qw[None, :]).T)
    rkT = np.ascontiguousarray((R * kw[None, :]).T)
    rots = np.stack([rqT, rkT], axis=1).astype(bf)         # [D, 2, D]

    tabs = np.stack([
        cosT * qw[:, None] * scale,
        sinT * scale,
        cosT * kw[:, None],
        sinT,
    ], axis=1).astype(bf)                                  # [D, 4, S]

    r = np.arange(P)[:, None]
    c = np.arange(P)[None, :]
    dmask = np.where(c >= r, 1.0, 0.0)                     # [k, q]: q >= k
    emask = np.where(c < r, 1.0, 0.0)                      # [k, q]: q < k
    msk = np.stack([dmask, emask], axis=1).astype(bf)      # [128, 2, 128]

    def pack_w(w_slice):
        # [O, H] -> lhsT [H, O] -> [128, NHT, O]
        wT = w_slice.T.astype(np.float64)
        O = wT.shape[1]
        return np.ascontiguousarray(
            wT.reshape(NHT, P, O).transpose(1, 0, 2)).astype(bf)

    hs_packed = []
    for b in range(B):
        hsT = hidden_states[b].T.astype(np.float64)        # [H, S]
        # quarter-major layout [p, sc, q, ht, s128]
        hs5 = hsT.reshape(NHT, P, NCH, 4, P).transpose(1, 2, 3, 0, 4)
        hs_packed.append(np.ascontiguousarray(hs5).astype(bf))

    in_maps = []
    for core in range(8):
        b, g = divmod(core, 4)
        woT = wo[:, 512 * g:512 * (g + 1)].T.astype(np.float64)  # [512, H]
        wo_r = np.ascontiguousarray(
            woT.reshape(NQC, P, H).transpose(1, 0, 2)).astype(bf)
        in_maps.append({
            "hs": hs_packed[b],
            "wq": pack_w(wq[512 * g:512 * (g + 1), :]),
            "wk": pack_w(wk[256 * g:256 * (g + 1), :]),
            "wv": pack_w(wv[256 * g:256 * (g + 1), :]),
            "wo": wo_r,
            "tabs": tabs, "rots": rots, "msk": msk,
        })
    return in_maps


def _postprocess(results):
    out = np.empty((B, S, H), np.float32)
    for b in range(B):
        acc = np.zeros((H, S), np.float32)
        for g in range(4):
            y_r = results[4 * b + g]["y"].astype(np.float32)  # [128, 16, S]
            acc += y_r.transpose(1, 0, 2).reshape(H, S)
        out[b] = acc.T
    return out


def kernel(hidden_states, wq, wk, wv, wo, q_norm_weight, k_norm_weight):
    nc = _build_nc()
    in_maps = _host_inputs(hidden_states, wq, wk, wv, wo,
                           q_norm_weight, k_norm_weight)
    res = run_bass_kernel_spmd(nc, in_maps, list(range(8)))
    return _postprocess(res.results)


# revision 30
# speedup vs baseline: 1.4063x; 1.0866x over previous
"""Gemma3 sliding-window attention on 8 Trainium2 NeuronCores.

Sharding: core c handles batch b=c//4 and head-group g=c%4 (4 of 16 q heads,
2 of 8 kv heads). wq/wk/wv column-split, wo row-split; the 4 partial outputs
per batch are summed on host (no device collectives).

v2: all matmul operands in bf16 (fp32 PSUM accumulation), single-instruction
batched DMA loads from host-prepacked layouts, per-q-tile(128) attention with
the softmax denominator computed as a 129th V column in a [q,d]-oriented PV
matmul, XBAR DMA transposes to return attn to [d,q] for the output
projection, and software-pipelined instruction issue so the PE never waits
on the RMSNorm/RoPE vector chains.
"""

import math
import numpy as np
import ml_dtypes

import concourse.bacc as bacc
import concourse.mybir as mybir
import concourse.tile as tile
from concourse.bass_utils import run_bass_kernel_spmd

dt = mybir.dt
AFT = mybir.ActivationFunctionType
BF = dt.bfloat16
F32 = dt.float32

B, S, H = 2, 2048, 2048
NQC, NKVC, D = 4, 2, 128          # per-core heads
WIN = 1024
EPS = 1e-6
THETA = 10000.0
P = 128
SCP = 512                          # phase-1 seq chunk
NCH = S // SCP                     # 4
NHT = H // P                       # 16
NST = S // P                       # 16
WT = WIN // P                      # 8 (window in tiles)
LAG = 4                            # attention PV pipeline depth (units)

_CACHE = {}


def _groups_for(t0, u0):
    """k-tile groups for one q tile: runs of <=4 tiles, diagonal tile alone
    last (so its [128,128] exp/mask stays separate)."""
    ts = list(range(t0, u0 + 1))
    if len(ts) == 1:
        return [ts]
    body, diag = ts[:-1], ts[-1:]
    gs = [body[i:i + 4] for i in range(0, len(body), 4)]
    gs.append(diag)
    return gs


def _build_nc():
    if "nc" in _CACHE:
        return _CACHE["nc"]
    nc = bacc.Bacc("TRN2", target_bir_lowering=False, debug=False, num_devices=8)

    F8 = dt.float8e4
    DR = mybir.MatmulPerfMode.DoubleRow
    # hi/lo fp8 pairs: x ~= hi + lo to ~0.1% rms; DoubleRow matmuls run the
    # (hi,hi), (hi,lo), (lo,hi) cross terms at 0.5 cyc/row over ht-pairs.
    hs_d = nc.dram_tensor("hs", [P, NCH, 2, NHT // 2, 2, 4, P], F8,
                          kind="ExternalInput").ap()
    wq_d = nc.dram_tensor("wq", [P, 2, NHT // 2, 2, NQC * D], F8,
                          kind="ExternalInput").ap()
    wk_d = nc.dram_tensor("wk", [P, 2, NHT // 2, 2, NKVC * D], F8,
                          kind="ExternalInput").ap()
    wv_d = nc.dram_tensor("wv", [P, 2, NHT // 2, 2, NKVC * D], F8,
                          kind="ExternalInput").ap()
    wo_d = nc.dram_tensor("wo", [P, NQC, H], BF, kind="ExternalInput").ap()
    tabs_d = nc.dram_tensor("tabs", [P, 4, S], BF, kind="ExternalInput").ap()
    rots_d = nc.dram_tensor("rots", [P, 2, P], BF, kind="ExternalInput").ap()
    msk_d = nc.dram_tensor("msk", [P, 2, P], BF, kind="ExternalInput").ap()
    y_d = nc.dram_tensor("y", [P, NHT, S], BF, kind="ExternalOutput").ap()

    with nc.allow_low_precision(reason="bf16 kernel; rel-err budget 2e-2"), \
         tile.TileContext(nc) as tc:
        with (
            tc.tile_pool(name="const", bufs=1) as cpool,
            tc.tile_pool(name="qkv", bufs=1) as qkv,
            tc.tile_pool(name="wts", bufs=1) as wts,
        ):
            msk_sb = cpool.tile([P, 2, P], BF, tag="msk")
            rots_sb = cpool.tile([P, 2, P], BF, tag="rots")
            ones_sb = cpool.tile([P, P], BF, tag="ones")
            eps_sb = cpool.tile([P, 1], F32, tag="eps")
            nc.vector.memset(ones_sb[:], 1.0)
            nc.vector.memset(eps_sb[:], EPS)
            dm_sb = msk_sb[:, 0, :]
            em_sb = msk_sb[:, 1, :]

            # weight loads: wv first (v-projection is the startup filler work),
            # then wk (k heads run before q heads), wq, wo last-needed.
            wv_sb = wts.tile([P, 2, NHT // 2, 2, NKVC * D], F8, tag="wv")
            wk_sb = wts.tile([P, 2, NHT // 2, 2, NKVC * D], F8, tag="wk")
            wq_sb = wts.tile([P, 2, NHT // 2, 2, NQC * D], F8, tag="wq")
            wo_sb = wts.tile([P, NQC, H], BF, tag="wo")

            qn_sb = qkv.tile([P, NQC, S], BF, tag="qn")
            kn_sb = qkv.tile([P, NKVC, S], BF, tag="kn")
            v_sb = qkv.tile([P, NST, NKVC, D + 1], BF, tag="v")
            nc.vector.memset(v_sb[:, :, :, D:D + 1], 64.0)

            # ---------------- phase 1: QKV projections + RMSNorm + RoPE ----
            # per (chunk, head): PE proj chain -> Act copy -> DVE square /
            # rope muls; the sum-of-squares and rotation matmuls for head m
            # are issued after head m+1's projection chain so PE never waits.
            with (
                tc.tile_pool(name="hsp", bufs=2) as hsp,
                tc.tile_pool(name="tabp", bufs=2) as tabp,
                tc.tile_pool(name="cpp", bufs=2) as cpp,
                tc.tile_pool(name="t1", bufs=2) as t1p,
                tc.tile_pool(name="t2", bufs=2) as t2p,
                tc.tile_pool(name="t3", bufs=2) as t3p,
                tc.tile_pool(name="t4", bufs=2) as t4p,
                tc.tile_pool(name="t5", bufs=2) as t5p,
                tc.tile_pool(name="t6", bufs=2) as t6p,
                tc.tile_pool(name="pp", bufs=2, space="PSUM") as ppp,
                tc.tile_pool(name="prb", bufs=2, space="PSUM") as prbp,
                tc.tile_pool(name="pvb", bufs=2, space="PSUM") as pvbp,
                tc.tile_pool(name="psv", bufs=2, space="PSUM") as psvp,
            ):
                # heads order: k0, k1, q0..q3 (wk arrives before wq)
                HEADS = [("k", 0), ("k", 1), ("q", 0), ("q", 1), ("q", 2), ("q", 3)]
                pend = None  # deferred norm/rope finish of the previous head

                def proj_chain(out_ps, w_sb8, mcols, hs_t):
                    first = True
                    for wi, xi in ((0, 0), (0, 1), (1, 0)):
                        for tp in range(NHT // 2):
                            nc.tensor.matmul(
                                out_ps[:], w_sb8[:, wi, tp, :, mcols],
                                hs_t[:, xi, tp, :, :, :],
                                perf_mode=DR, start=first,
                                stop=(wi == 1 and tp == NHT // 2 - 1))
                            first = False

                def v_chain(out_ps, hs_t, ss):
                    first = True
                    for wi, xi in ((0, 0), (0, 1), (1, 0)):
                        for tp in range(NHT // 2):
                            nc.tensor.matmul(
                                out_ps[:], hs_t[:, xi, tp, :, ss, :],
                                wv_sb[:, wi, tp, :, :],
                                perf_mode=DR, start=first,
                                stop=(wi == 1 and tp == NHT // 2 - 1))
                            first = False

                def finish(st):
                    kind, m, pp, cp, u_t, s0, tab_t = st
                    sq = t1p.tile([P, SCP], BF, tag="sq")
                    nc.vector.tensor_mul(sq[:], cp[:], cp[:])
                    rb = prbp.tile([P, SCP], F32, tag="rb")
                    rot = rots_sb[:, 0, :] if kind == "q" else rots_sb[:, 1, :]
                    nc.tensor.matmul(rb[:], rot, cp[:], start=True, stop=True)
                    vb = pvbp.tile([P, SCP], F32, tag="vb")
                    nc.tensor.matmul(vb[:], ones_sb[:], sq[:], start=True, stop=True)
                    sd = t2p.tile([P, SCP], F32, tag="sd")
                    nc.scalar.activation(sd[:], vb[:], AFT.Sqrt, bias=eps_sb[:],
                                         scale=1.0 / D)
                    inv = t3p.tile([P, SCP], BF, tag="inv")
                    nc.vector.reciprocal(inv[:], sd[:])
                    # tsin: rb (PSUM) is ready late
                    tsin = t4p.tile([P, SCP], BF, tag="tsin")
                    sin_t = tab_t[:, 1 if kind == "q" else 3, :]
                    nc.vector.tensor_mul(tsin[:], rb[:], sin_t)
                    nc.vector.tensor_add(u_t[:], u_t[:], tsin[:])
                    dst = qn_sb if kind == "q" else kn_sb
                    nc.vector.tensor_mul(dst[:, m, s0:s0 + SCP], u_t[:], inv[:])

                for sc in range(NCH):
                    s0 = sc * SCP
                    hs_sb = hsp.tile([P, 2, NHT // 2, 2, 4, P], F8, tag="hs")
                    if sc == 0:
                        # startup-critical order: hi parts first (the hi-hi
                        # chain leads each accumulation), v before k/q.
                        nc.sync.dma_start(out=wv_sb[:, 0], in_=wv_d[:, 0])
                        nc.sync.dma_start(out=hs_sb[:, 0], in_=hs_d[:, 0, 0])
                        nc.sync.dma_start(out=wv_sb[:, 1], in_=wv_d[:, 1])
                        nc.sync.dma_start(out=hs_sb[:, 1], in_=hs_d[:, 0, 1])
                        nc.sync.dma_start(out=wk_sb[:], in_=wk_d[:])
                        nc.sync.dma_start(out=msk_sb[:], in_=msk_d[:])
                        nc.sync.dma_start(out=rots_sb[:], in_=rots_d[:])
                        nc.sync.dma_start(out=wq_sb[:], in_=wq_d[:])
                    else:
                        nc.sync.dma_start(out=hs_sb[:], in_=hs_d[:, sc])
                    tab_sb = tabp.tile([P, 4, SCP], BF, tag="tab")
                    nc.sync.dma_start(out=tab_sb[:], in_=tabs_d[:, :, s0:s0 + SCP])
                    if sc == 0:
                        nc.sync.dma_start(out=wo_sb[:], in_=wo_d[:])

                    # v projection: natural [seq, d] layout + filler work
                    for ss in range(SCP // P):
                        st_g = sc * (SCP // P) + ss
                        pv = psvp.tile([P, NKVC * D], F32, tag="pv")
                        v_chain(pv, hs_sb, ss)
                        nc.vector.tensor_copy(v_sb[:, st_g, :, 0:D], pv[:])

                    for kind, m in HEADS:
                        w_sb = wq_sb if kind == "q" else wk_sb
                        pp = ppp.tile([P, SCP], F32, tag="pp")
                        proj_chain(pp, w_sb, slice(m * D, (m + 1) * D), hs_sb)
                        cp = cpp.tile([P, SCP], BF, tag="cp")
                        nc.scalar.copy(cp[:], pp[:])
                        u_t = t5p.tile([P, SCP], BF, tag="u")
                        cos_t = tab_sb[:, 0 if kind == "q" else 2, :]
                        nc.vector.tensor_mul(u_t[:], cp[:], cos_t)
                        if pend is not None:
                            finish(pend)
                        pend = (kind, m, pp, cp, u_t, s0, tab_sb)
                finish(pend)

            # ---------------- phase 2+3: attention + output projection -----
            with (
                tc.tile_pool(name="pb", bufs=LAG + 2) as pbp,
                tc.tile_pool(name="invp", bufs=4) as invp,
                tc.tile_pool(name="aq", bufs=2) as aqp,
                tc.tile_pool(name="aT", bufs=2) as aTp,
                tc.tile_pool(name="ysb", bufs=2) as ysp,
                tc.tile_pool(name="psc", bufs=4, space="PSUM") as pscp,
                tc.tile_pool(name="pa", bufs=2, space="PSUM") as pap,
                tc.tile_pool(name="psy", bufs=2, space="PSUM") as psyp,
            ):
                queue = []
                slab_tiles = {}

                def emit_scores(h, u0):
                    kvh = h // 2
                    t0 = max(0, u0 - WT)
                    gs = _groups_for(t0, u0)
                    p_t = pbp.tile([P, (WT + 1) * P], BF, tag="p")
                    for g in gs:
                        sc_t = pscp.tile([P, 4 * P], F32, tag="sc")
                        for i, t in enumerate(g):
                            nc.tensor.matmul(
                                sc_t[:, i * P:(i + 1) * P],
                                kn_sb[:, kvh, t * P:(t + 1) * P],
                                qn_sb[:, h, u0 * P:(u0 + 1) * P],
                                start=True, stop=True)
                        off = (g[0] - t0) * P
                        n = len(g) * P
                        nc.scalar.activation(p_t[:, off:off + n], sc_t[:, 0:n],
                                             AFT.Exp)
                        for t in g:
                            # masks on the otherwise-idle GPSIMD engine: DVE
                            # runs ~90% busy in this phase and its queue
                            # latency was stalling the PV chains
                            if t == u0:
                                blk = p_t[:, (t - t0) * P:(t - t0 + 1) * P]
                                nc.vector.tensor_mul(blk, blk, dm_sb)
                            elif u0 >= WT and t == u0 - WT:
                                blk = p_t[:, (t - t0) * P:(t - t0 + 1) * P]
                                nc.vector.tensor_mul(blk, blk, em_sb)
                    return (h, u0, t0, p_t)

                def emit_pv(st):
                    h, u0, t0, p_t = st
                    kvh = h // 2
                    ts = list(range(t0, u0 + 1))
                    a_t = pap.tile([P, D + 1], F32, tag="a")
                    for i, t in enumerate(ts):
                        nc.tensor.matmul(
                            a_t[:], p_t[:, i * P:(i + 1) * P],
                            v_sb[:, t, kvh, :],
                            start=(i == 0), stop=(i == len(ts) - 1))
                    inv = invp.tile([P, 1], F32, tag="inv")
                    nc.vector.reciprocal(inv[:], a_t[:, D:D + 1])
                    slab = slab_tiles[u0 // 4]
                    nc.vector.tensor_scalar_mul(slab[:, h, u0 % 4, :],
                                                a_t[:, 0:D], inv[:])

                def emit_transpose(s4, h):
                    # Act HWDGE queue: avoids head-of-line blocking behind the
                    # SP queue's y-out DMAs (which wait on DVE copies). Issued
                    # one per unit so the Act SEQ time (~0.7us per DMA) does
                    # not delay exp dispatch in a lump.
                    if h == 0:
                        aT = aTp.tile([P, NQC, 4, P], BF, tag="aT", name="aT")
                        slab_tiles[("T", s4)] = aT
                    aT = slab_tiles[("T", s4)]
                    slab = slab_tiles[s4]
                    nc.scalar.dma_start_transpose(out=aT[:, h, :, :],
                                                  in_=slab[:, h, :, :])

                def emit_outproj(s4):
                    aT = slab_tiles[("T", s4)]
                    for mog in range(4):
                        y_t = ysp.tile([P, 4, SCP], BF, tag="y")
                        for mo4 in range(4):
                            mo = mog * 4 + mo4
                            yp = psyp.tile([P, SCP], F32, tag="yp")
                            for h in range(NQC):
                                nc.tensor.matmul(
                                    yp[:], wo_sb[:, h, mo * P:(mo + 1) * P],
                                    aT[:, h, :, :],
                                    start=(h == 0), stop=(h == NQC - 1))
                            nc.vector.tensor_copy(y_t[:, mo4, :], yp[:])
                        nc.sync.dma_start(
                            out=y_d[:, mog * 4:(mog + 1) * 4,
                                    s4 * SCP:(s4 + 1) * SCP],
                            in_=y_t[:])

                for u0 in range(NST):
                    if u0 % 4 == 0:
                        slab_tiles[u0 // 4] = aqp.tile([P, NQC, 4, P], BF,
                                                       tag="aq", name="aq")
                    if u0 % 4 == 3 and u0 >= 7:
                        emit_outproj((u0 - 7) // 4)
                    for h in range(NQC):
                        queue.append(emit_scores(h, u0))
                        if len(queue) > LAG:
                            emit_pv(queue.pop(0))
                        if u0 == NST - 1 and queue:
                            emit_pv(queue.pop(0))  # drain early for the tail
                        if u0 % 4 == 1 and u0 >= 5:
                            emit_transpose((u0 - 5) // 4, h)
                while queue:
                    emit_pv(queue.pop(0))
                # last slab: split transposes across both HWDGE queues to
                # halve the serial latency in the tail
                aT = aTp.tile([P, NQC, 4, P], BF, tag="aT", name="aT")
                slab_tiles[("T", 3)] = aT
                slab = slab_tiles[3]
                for h in range(NQC):
                    eng = nc.scalar if h % 2 == 0 else nc.sync
                    eng.dma_start_transpose(out=aT[:, h, :, :],
                                            in_=slab[:, h, :, :])
                emit_outproj(3)

    nc.compile()
    _CACHE["nc"] = nc
    return nc


def _host_inputs(hidden_states, wq, wk, wv, wo, q_norm_weight, k_norm_weight):
    """Per-core input dicts (8 cores: c = 4*b + g)."""
    bf = ml_dtypes.bfloat16
    f = np.float32
    scale = 1.0 / math.sqrt(D)
    inv_freq = 1.0 / (THETA ** (np.arange(0, D, 2, dtype=np.float64) / D))
    t = np.arange(S, dtype=np.float64)
    freqs = np.outer(t, inv_freq)
    emb = np.concatenate([freqs, freqs], axis=-1)          # [S, D]
    cosT = np.cos(emb).T.astype(np.float64)                # [D, S]
    sinT = np.sin(emb).T.astype(np.float64)
    qw = (1.0 + q_norm_weight).astype(np.float64)
    kw = (1.0 + k_norm_weight).astype(np.float64)

    # rotate-half matrices with norm weights folded (lhsT layout, like
    # baseline): rb = rots.T @ x = (R * w) @ x
    R = np.zeros((D, D), np.float64)
    hh = D // 2
    for i in range(hh):
        R[i, i + hh] = -1.0
        R[i + hh, i] = 1.0
    rqT = np.ascontiguousarray((R * qw[None, :]).T)
    rkT = np.ascontiguousarray((R * kw[None, :]).T)
    rots = np.stack([rqT, rkT], axis=1).astype(bf)         # [D, 2, D]

    tabs = np.stack([
        cosT * qw[:, None] * scale,
        sinT * scale,
        cosT * kw[:, None],
        sinT,
    ], axis=1).astype(bf)                                  # [D, 4, S]

    r = np.arange(P)[:, None]
    c = np.arange(P)[None, :]
    dmask = np.where(c >= r, 1.0, 0.0)                     # [k, q]: q >= k
    emask = np.where(c < r, 1.0, 0.0)                      # [k, q]: q < k
    msk = np.stack([dmask, emask], axis=1).astype(bf)      # [128, 2, 128]

    f8 = ml_dtypes.float8_e4m3
    WS = 64.0  # weight scale: fp8 sweet spot; absorbed by RMSNorm (q/k) and
               # by the 64-valued ones-column of v_ext (v)

    def hi_lo(x):
        hi = x.astype(f8)
        lo = (x - hi.astype(np.float64)).astype(f8)
        return np.stack([hi, lo], axis=1)

    def pack_w(w_slice):
        # [O, H] -> lhsT [H, O] -> [128, 2(hl), 8(tp), 2(ti), O] fp8 x64
        wT = w_slice.T.astype(np.float64) * WS
        O = wT.shape[1]
        base = wT.reshape(NHT // 2, 2, P, O).transpose(2, 0, 1, 3)
        return np.ascontiguousarray(hi_lo(base))

    hs_packed = []
    for b in range(B):
        hsT = hidden_states[b].T.astype(np.float64)        # [H, S]
        # [p, sc, 2(hl), 8(tp), 2(ti), 4(q), s128] fp8 hi/lo
        hs6 = hsT.reshape(NHT // 2, 2, P, NCH, 4, P).transpose(2, 3, 0, 1, 4, 5)
        hi = hs6.astype(f8)
        lo = (hs6 - hi.astype(np.float64)).astype(f8)
        hs_packed.append(np.ascontiguousarray(np.stack([hi, lo], axis=2)))

    in_maps = []
    for core in range(8):
        b, g = divmod(core, 4)
        woT = wo[:, 512 * g:512 * (g + 1)].T.astype(np.float64)  # [512, H]
        wo_r = np.ascontiguousarray(
            woT.reshape(NQC, P, H).transpose(1, 0, 2)).astype(bf)
        in_maps.append({
            "hs": hs_packed[b],
            "wq": pack_w(wq[512 * g:512 * (g + 1), :]),
            "wk": pack_w(wk[256 * g:256 * (g + 1), :]),
            "wv": pack_w(wv[256 * g:256 * (g + 1), :]),
            "wo": wo_r,
            "tabs": tabs, "rots": rots, "msk": msk,
        })
    return in_maps


def _postprocess(results):
    out = np.empty((B, S, H), np.float32)
    for b in range(B):
        acc = np.zeros((H, S), np.float32)
        for g in range(4):
            y_r = results[4 * b + g]["y"].astype(np.float32)  # [128, 16, S]
            acc += y_r.transpose(1, 0, 2).reshape(H, S)
        out[b] = acc.T
    return out


def kernel(hidden_states, wq, wk, wv, wo, q_norm_weight, k_norm_weight):
    nc = _build_nc()
    in_maps = _host_inputs(hidden_states, wq, wk, wv, wo,
                           q_norm_weight, k_norm_weight)
    res = run_bass_kernel_spmd(nc, in_maps, list(range(8)))
    return _postprocess(res.results)


# revision 40
# speedup vs baseline: 1.4764x; 1.0499x over previous
"""Gemma3 sliding-window attention on 8 Trainium2 NeuronCores.

Sharding: core c handles batch b=c//4 and head-group g=c%4 (4 of 16 q heads,
2 of 8 kv heads). wq/wk/wv column-split, wo row-split; the 4 partial outputs
per batch are summed on host (no device collectives).

v2: all matmul operands in bf16 (fp32 PSUM accumulation), single-instruction
batched DMA loads from host-prepacked layouts, per-q-tile(128) attention with
the softmax denominator computed as a 129th V column in a [q,d]-oriented PV
matmul, XBAR DMA transposes to return attn to [d,q] for the output
projection, and software-pipelined instruction issue so the PE never waits
on the RMSNorm/RoPE vector chains.
"""

import math
import numpy as np
import ml_dtypes

import concourse.bacc as bacc
import concourse.mybir as mybir
import concourse.tile as tile
from concourse.bass_utils import run_bass_kernel_spmd

dt = mybir.dt
AFT = mybir.ActivationFunctionType
BF = dt.bfloat16
F32 = dt.float32

B, S, H = 2, 2048, 2048
NQC, NKVC, D = 4, 2, 128          # per-core heads
WIN = 1024
EPS = 1e-6
THETA = 10000.0
P = 128
SCP = 512                          # phase-1 seq chunk
NCH = S // SCP                     # 4
NHT = H // P                       # 16
NST = S // P                       # 16
WT = WIN // P                      # 8 (window in tiles)
LAG = 2                            # attention PV pipeline depth (pair units)

_CACHE = {}


def _groups_for(t0, u0):
    """k-tile groups for one q tile: runs of <=4 tiles, diagonal tile alone
    last (so its [128,128] exp/mask stays separate)."""
    ts = list(range(t0, u0 + 1))
    if len(ts) == 1:
        return [ts]
    body, diag = ts[:-1], ts[-1:]
    gs = [body[i:i + 4] for i in range(0, len(body), 4)]
    gs.append(diag)
    return gs


def _build_nc():
    if "nc" in _CACHE:
        return _CACHE["nc"]
    nc = bacc.Bacc("TRN2", target_bir_lowering=False, debug=False, num_devices=8)

    F8 = dt.float8e4
    DR = mybir.MatmulPerfMode.DoubleRow
    # hi/lo fp8 pairs: x ~= hi + lo to ~0.1% rms; DoubleRow matmuls run the
    # (hi,hi), (hi,lo), (lo,hi) cross terms at 0.5 cyc/row over ht-pairs.
    hs_d = nc.dram_tensor("hs", [P, NCH, 2, NHT // 2, 2, 4, P], F8,
                          kind="ExternalInput").ap()
    wq_d = nc.dram_tensor("wq", [P, 2, NHT // 2, 2, NQC * D], F8,
                          kind="ExternalInput").ap()
    wk_d = nc.dram_tensor("wk", [P, 2, NHT // 2, 2, NKVC * D], F8,
                          kind="ExternalInput").ap()
    wv_d = nc.dram_tensor("wv", [P, 2, NHT // 2, 2, NKVC * D], F8,
                          kind="ExternalInput").ap()
    wo_d = nc.dram_tensor("wo", [P, NQC, H], BF, kind="ExternalInput").ap()
    tabs_d = nc.dram_tensor("tabs", [P, 4, S], BF, kind="ExternalInput").ap()
    rots_d = nc.dram_tensor("rots", [P, 2, P], BF, kind="ExternalInput").ap()
    msk_d = nc.dram_tensor("msk", [P, 2, 2, P], BF, kind="ExternalInput").ap()
    y_d = nc.dram_tensor("y", [P, NHT, S], BF, kind="ExternalOutput").ap()

    with nc.allow_low_precision(reason="bf16 kernel; rel-err budget 2e-2"), \
         tile.TileContext(nc) as tc:
        with (
            tc.tile_pool(name="const", bufs=1) as cpool,
            tc.tile_pool(name="qkv", bufs=1) as qkv,
            tc.tile_pool(name="wts", bufs=1) as wts,
        ):
            msk_sb = cpool.tile([P, 2, 2, P], BF, tag="msk")
            rots_sb = cpool.tile([P, 2, P], BF, tag="rots")
            ones_sb = cpool.tile([P, P], BF, tag="ones")
            eps_sb = cpool.tile([P, 1], F32, tag="eps")
            nc.vector.memset(ones_sb[:], 1.0)
            nc.vector.memset(eps_sb[:], EPS)
            dm_sb = msk_sb[:, 0, :, :]
            em_sb = msk_sb[:, 1, :, :]

            # weight loads: wv first (v-projection is the startup filler work),
            # then wk (k heads run before q heads), wq, wo last-needed.
            wv_sb = wts.tile([P, 2, NHT // 2, 2, NKVC * D], F8, tag="wv")
            wk_sb = wts.tile([P, 2, NHT // 2, 2, NKVC * D], F8, tag="wk")
            wq_sb = wts.tile([P, 2, NHT // 2, 2, NQC * D], F8, tag="wq")
            wo_sb = wts.tile([P, NQC, H], BF, tag="wo")

            qn_sb = qkv.tile([P, NQC, S], BF, tag="qn")
            kn_sb = qkv.tile([P, NKVC, S], BF, tag="kn")
            v_sb = qkv.tile([P, NST, NKVC, D + 1], BF, tag="v")
            nc.vector.memset(v_sb[:, :, :, D:D + 1], 64.0)

            # ---------------- phase 1: QKV projections + RMSNorm + RoPE ----
            # per (chunk, head): PE proj chain -> Act copy -> DVE square /
            # rope muls; the sum-of-squares and rotation matmuls for head m
            # are issued after head m+1's projection chain so PE never waits.
            with (
                tc.tile_pool(name="hsp", bufs=2) as hsp,
                tc.tile_pool(name="tabp", bufs=2) as tabp,
                tc.tile_pool(name="cpp", bufs=4) as cpp,
                tc.tile_pool(name="t1", bufs=3) as t1p,
                tc.tile_pool(name="t2", bufs=3) as t2p,
                tc.tile_pool(name="t3", bufs=3) as t3p,
                tc.tile_pool(name="t4", bufs=3) as t4p,
                tc.tile_pool(name="t5", bufs=4) as t5p,
                tc.tile_pool(name="t6", bufs=2) as t6p,
                tc.tile_pool(name="pp", bufs=2, space="PSUM") as ppp,
                tc.tile_pool(name="prb", bufs=2, space="PSUM") as prbp,
                tc.tile_pool(name="pvb", bufs=2, space="PSUM") as pvbp,
                tc.tile_pool(name="psv", bufs=2, space="PSUM") as psvp,
            ):
                # heads order: k0, k1, q0..q3 (wk arrives before wq)
                HEADS = [("k", 0), ("k", 1), ("q", 0), ("q", 1), ("q", 2), ("q", 3)]
                pend = []  # deferred norm/rope finishes (2-deep pipeline)

                def proj_chain(out_ps, w_sb8, mcols, hs_t):
                    first = True
                    for wi, xi in ((0, 0), (0, 1), (1, 0)):
                        for tp in range(NHT // 2):
                            nc.tensor.matmul(
                                out_ps[:], w_sb8[:, wi, tp, :, mcols],
                                hs_t[:, xi, tp, :, :, :],
                                perf_mode=DR, start=first,
                                stop=(wi == 1 and tp == NHT // 2 - 1))
                            first = False

                def v_chain(out_ps, hs_t, ss):
                    first = True
                    for wi, xi in ((0, 0), (0, 1), (1, 0)):
                        for tp in range(NHT // 2):
                            nc.tensor.matmul(
                                out_ps[:], hs_t[:, xi, tp, :, ss, :],
                                wv_sb[:, wi, tp, :, :],
                                perf_mode=DR, start=first,
                                stop=(wi == 1 and tp == NHT // 2 - 1))
                            first = False

                def finish(st):
                    kind, m, pp, cp, u_t, s0, tab_t = st
                    sq = t1p.tile([P, SCP], BF, tag="sq")
                    nc.vector.tensor_mul(sq[:], cp[:], cp[:])
                    rb = prbp.tile([P, SCP], F32, tag="rb")
                    rot = rots_sb[:, 0, :] if kind == "q" else rots_sb[:, 1, :]
                    nc.tensor.matmul(rb[:], rot, cp[:], start=True, stop=True)
                    vb = pvbp.tile([P, SCP], F32, tag="vb")
                    nc.tensor.matmul(vb[:], ones_sb[:], sq[:], start=True, stop=True)
                    sd = t2p.tile([P, SCP], F32, tag="sd")
                    nc.scalar.activation(sd[:], vb[:], AFT.Sqrt, bias=eps_sb[:],
                                         scale=1.0 / D)
                    inv = t3p.tile([P, SCP], BF, tag="inv")
                    nc.vector.reciprocal(inv[:], sd[:])
                    # tsin: rb (PSUM) is ready late
                    tsin = t4p.tile([P, SCP], BF, tag="tsin")
                    sin_t = tab_t[:, 1 if kind == "q" else 3, :]
                    nc.vector.tensor_mul(tsin[:], rb[:], sin_t)
                    nc.vector.tensor_add(u_t[:], u_t[:], tsin[:])
                    dst = qn_sb if kind == "q" else kn_sb
                    nc.vector.tensor_mul(dst[:, m, s0:s0 + SCP], u_t[:], inv[:])

                for sc in range(NCH):
                    s0 = sc * SCP
                    hs_sb = hsp.tile([P, 2, NHT // 2, 2, 4, P], F8, tag="hs")
                    if sc == 0:
                        # startup-critical order: hi parts first (the hi-hi
                        # chain leads each accumulation), v before k/q.
                        nc.sync.dma_start(out=wv_sb[:, 0], in_=wv_d[:, 0])
                        nc.sync.dma_start(out=hs_sb[:, 0, 0:4], in_=hs_d[:, 0, 0, 0:4])
                        nc.sync.dma_start(out=hs_sb[:, 0, 4:8], in_=hs_d[:, 0, 0, 4:8])
                        nc.sync.dma_start(out=wv_sb[:, 1], in_=wv_d[:, 1])
                        nc.sync.dma_start(out=hs_sb[:, 1], in_=hs_d[:, 0, 1])
                        nc.sync.dma_start(out=wk_sb[:], in_=wk_d[:])
                        nc.sync.dma_start(out=msk_sb[:], in_=msk_d[:])
                        nc.sync.dma_start(out=rots_sb[:], in_=rots_d[:])
                        nc.sync.dma_start(out=wq_sb[:], in_=wq_d[:])
                    else:
                        nc.sync.dma_start(out=hs_sb[:], in_=hs_d[:, sc])
                    tab_sb = tabp.tile([P, 4, SCP], BF, tag="tab")
                    nc.sync.dma_start(out=tab_sb[:], in_=tabs_d[:, :, s0:s0 + SCP])
                    if sc == 0:
                        nc.sync.dma_start(out=wo_sb[:], in_=wo_d[:])

                    # v projection: natural [seq, d] layout + filler work
                    for ss in range(SCP // P):
                        st_g = sc * (SCP // P) + ss
                        pv = psvp.tile([P, NKVC * D], F32, tag="pv")
                        v_chain(pv, hs_sb, ss)
                        nc.vector.tensor_copy(v_sb[:, st_g, :, 0:D], pv[:])
                        if ss == 0 and pend:
                            # finish the previous chunk's last head here: its
                            # DVE ops land ahead of the remaining v copies
                            finish(pend.pop(0))

                    for kind, m in HEADS:
                        w_sb = wq_sb if kind == "q" else wk_sb
                        pp = ppp.tile([P, SCP], F32, tag="pp")
                        proj_chain(pp, w_sb, slice(m * D, (m + 1) * D), hs_sb)
                        cp = cpp.tile([P, SCP], BF, tag="cp")
                        nc.scalar.copy(cp[:], pp[:])
                        u_t = t5p.tile([P, SCP], BF, tag="u")
                        cos_t = tab_sb[:, 0 if kind == "q" else 2, :]
                        nc.vector.tensor_mul(u_t[:], cp[:], cos_t)
                        pend.append((kind, m, pp, cp, u_t, s0, tab_sb))
                        if len(pend) > 1:
                            finish(pend.pop(0))
                for st in pend:
                    finish(st)

            # ---------------- phase 2+3: attention + output projection -----
            with (
                tc.tile_pool(name="pb", bufs=LAG + 2) as pbp,
                tc.tile_pool(name="invp", bufs=4) as invp,
                tc.tile_pool(name="aq", bufs=2) as aqp,
                tc.tile_pool(name="aT", bufs=2) as aTp,
                tc.tile_pool(name="ysb", bufs=2) as ysp,
                tc.tile_pool(name="psc", bufs=4, space="PSUM") as pscp,
                tc.tile_pool(name="pa", bufs=2, space="PSUM") as pap,
                tc.tile_pool(name="psy", bufs=2, space="PSUM") as psyp,
            ):
                queue = []
                slab_tiles = {}

                def emit_scores(kvh, u0):
                    # paired unit: both q heads of this kv head at once
                    h0 = 2 * kvh
                    t0 = max(0, u0 - WT)
                    n = u0 - t0 + 1
                    p_t = pbp.tile([P, WT + 1, 2, P], BF, tag="p", name="p_t")
                    qn_sl = qn_sb[:, h0:h0 + 2, u0 * P:(u0 + 1) * P]
                    for g0 in range(0, n, 2):
                        gn = min(2, n - g0)
                        sc_t = pscp.tile([P, 2, 2, P], F32, tag="sc",
                                         name="sc_t")
                        for i in range(gn):
                            t = t0 + g0 + i
                            nc.tensor.matmul(
                                sc_t[:, i, :, :],
                                kn_sb[:, kvh, t * P:(t + 1) * P],
                                qn_sl, start=True, stop=True)
                        nc.scalar.activation(p_t[:, g0:g0 + gn, :, :],
                                             sc_t[:, 0:gn, :, :], AFT.Exp)
                        for i in range(gn):
                            t = t0 + g0 + i
                            if t == u0:
                                blk = p_t[:, g0 + i, :, :]
                                nc.vector.tensor_mul(blk, blk, dm_sb)
                            elif u0 >= WT and t == u0 - WT:
                                blk = p_t[:, g0 + i, :, :]
                                nc.vector.tensor_mul(blk, blk, em_sb)
                    return (kvh, u0, t0, p_t)

                def emit_pv(st):
                    kvh, u0, t0, p_t = st
                    n = u0 - t0 + 1
                    slab = slab_tiles[u0 // 4]
                    for hh in range(2):
                        h = 2 * kvh + hh
                        a_t = pap.tile([P, D + 1], F32, tag="a")
                        for i in range(n):
                            nc.tensor.matmul(
                                a_t[:], p_t[:, i, hh, :],
                                v_sb[:, t0 + i, kvh, :],
                                start=(i == 0), stop=(i == n - 1))
                        inv = invp.tile([P, 1], F32, tag="inv")
                        nc.vector.reciprocal(inv[:], a_t[:, D:D + 1])
                        nc.vector.tensor_scalar_mul(slab[:, h, u0 % 4, :],
                                                    a_t[:, 0:D], inv[:])

                def emit_transpose(s4, h):
                    # Act HWDGE queue: avoids head-of-line blocking behind the
                    # SP queue's y-out DMAs (which wait on DVE copies). Issued
                    # one per unit so the Act SEQ time (~0.7us per DMA) does
                    # not delay exp dispatch in a lump.
                    if h == 0:
                        aT = aTp.tile([P, NQC, 4, P], BF, tag="aT", name="aT")
                        slab_tiles[("T", s4)] = aT
                    aT = slab_tiles[("T", s4)]
                    slab = slab_tiles[s4]
                    nc.sync.dma_start_transpose(out=aT[:, h, :, :],
                                                  in_=slab[:, h, :, :])

                op_queue = []  # (s4, mo) outproj chains, spread across units
                op_state = {}

                def emit_outproj_chain():
                    if not op_queue:
                        return
                    s4, mo = op_queue.pop(0)
                    aT = slab_tiles[("T", s4)]
                    mog, mo4 = divmod(mo, 4)
                    if mo4 == 0:
                        op_state["y"] = ysp.tile([P, 4, SCP], BF, tag="y",
                                                 name="y_t")
                    y_t = op_state["y"]
                    yp = psyp.tile([P, SCP], F32, tag="yp")
                    for h in range(NQC):
                        nc.tensor.matmul(
                            yp[:], wo_sb[:, h, mo * P:(mo + 1) * P],
                            aT[:, h, :, :],
                            start=(h == 0), stop=(h == NQC - 1))
                    nc.vector.tensor_copy(y_t[:, mo4, :], yp[:])
                    if mo4 == 3:
                        nc.sync.dma_start(
                            out=y_d[:, mog * 4:(mog + 1) * 4,
                                    s4 * SCP:(s4 + 1) * SCP],
                            in_=y_t[:])

                def emit_outproj(s4):
                    op_queue.extend((s4, mo) for mo in range(NHT))

                # descending u0: the big steady-state units come first and
                # prime the PV pipeline; the small ramp units land at the end
                # where the outproj slabs provide PE filler work.
                for u0 in range(NST - 1, -1, -1):
                    if u0 % 4 == 3:
                        slab_tiles[u0 // 4] = aqp.tile([P, NQC, 4, P], BF,
                                                       tag="aq", name="aq")
                    if u0 % 4 == 0 and u0 <= NST - 8:
                        emit_outproj(u0 // 4 + 1)
                    for kvh in range(NKVC):
                        queue.append(emit_scores(kvh, u0))
                        if len(queue) > LAG:
                            emit_pv(queue.pop(0))
                        if u0 == 0 and queue:
                            emit_pv(queue.pop(0))  # drain early for the tail
                        if u0 % 4 == 2 and u0 <= NST - 6:
                            emit_transpose(u0 // 4 + 1, 2 * kvh)
                            emit_transpose(u0 // 4 + 1, 2 * kvh + 1)
                        emit_outproj_chain()
                        emit_outproj_chain()
                while queue:
                    emit_pv(queue.pop(0))
                    emit_outproj_chain()
                    emit_outproj_chain()
                # first slab (last processed): split transposes across both
                # HWDGE queues to halve the serial latency in the tail
                aT = aTp.tile([P, NQC, 4, P], BF, tag="aT", name="aT")
                slab_tiles[("T", 0)] = aT
                slab = slab_tiles[0]
                for h in range(NQC):
                    eng = nc.scalar if h % 2 == 0 else nc.sync
                    eng.dma_start_transpose(out=aT[:, h, :, :],
                                            in_=slab[:, h, :, :])
                emit_outproj(0)
                while op_queue:
                    emit_outproj_chain()

    nc.compile()
    _CACHE["nc"] = nc
    return nc


def _host_inputs(hidden_states, wq, wk, wv, wo, q_norm_weight, k_norm_weight):
    """Per-core input dicts (8 cores: c = 4*b + g)."""
    bf = ml_dtypes.bfloat16
    f = np.float32
    scale = 1.0 / math.sqrt(D)
    inv_freq = 1.0 / (THETA ** (np.arange(0, D, 2, dtype=np.float64) / D))
    t = np.arange(S, dtype=np.float64)
    freqs = np.outer(t, inv_freq)
    emb = np.concatenate([freqs, freqs], axis=-1)          # [S, D]
    cosT = np.cos(emb).T.astype(np.float64)                # [D, S]
    sinT = np.sin(emb).T.astype(np.float64)
    qw = (1.0 + q_norm_weight).astype(np.float64)
    kw = (1.0 + k_norm_weight).astype(np.float64)

    # rotate-half matrices with norm weights folded (lhsT layout, like
    # baseline): rb = rots.T @ x = (R * w) @ x
    R = np.zeros((D, D), np.float64)
    hh = D // 2
    for i in range(hh):
        R[i, i + hh] = -1.0
        R[i + hh, i] = 1.0
    rqT = np.ascontiguousarray((R * qw[None, :]).T)
    rkT = np.ascontiguousarray((R * kw[None, :]).T)
    rots = np.stack([rqT, rkT], axis=1).astype(bf)         # [D, 2, D]

    tabs = np.stack([
        cosT * qw[:, None] * scale,
        sinT * scale,
        cosT * kw[:, None],
        sinT,
    ], axis=1).astype(bf)                                  # [D, 4, S]

    r = np.arange(P)[:, None]
    c = np.arange(P)[None, :]
    dmask = np.where(c >= r, 1.0, 0.0)                     # [k, q]: q >= k
    emask = np.where(c < r, 1.0, 0.0)                      # [k, q]: q < k
    msk = np.stack([dmask, dmask, emask, emask],
                   axis=1).reshape(P, 2, 2, P).astype(bf)  # [k, dm/em, hdup, q]

    f8 = ml_dtypes.float8_e4m3
    WS = 64.0  # weight scale: fp8 sweet spot; absorbed by RMSNorm (q/k) and
               # by the 64-valued ones-column of v_ext (v)

    def hi_lo(x):
        hi = x.astype(f8)
        lo = (x - hi.astype(np.float64)).astype(f8)
        return np.stack([hi, lo], axis=1)

    def pack_w(w_slice):
        # [O, H] -> lhsT [H, O] -> [128, 2(hl), 8(tp), 2(ti), O] fp8 x64
        wT = w_slice.T.astype(np.float64) * WS
        O = wT.shape[1]
        base = wT.reshape(NHT // 2, 2, P, O).transpose(2, 0, 1, 3)
        return np.ascontiguousarray(hi_lo(base))

    hs_packed = []
    for b in range(B):
        hsT = hidden_states[b].T.astype(np.float64)        # [H, S]
        # [p, sc, 2(hl), 8(tp), 2(ti), 4(q), s128] fp8 hi/lo
        hs6 = hsT.reshape(NHT // 2, 2, P, NCH, 4, P).transpose(2, 3, 0, 1, 4, 5)
        hi = hs6.astype(f8)
        lo = (hs6 - hi.astype(np.float64)).astype(f8)
        hs_packed.append(np.ascontiguousarray(np.stack([hi, lo], axis=2)))

    in_maps = []
    for core in range(8):
        b, g = divmod(core, 4)
        woT = wo[:, 512 * g:512 * (g + 1)].T.astype(np.float64)  # [512, H]
        wo_r = np.ascontiguousarray(
            woT.reshape(NQC, P, H).transpose(1, 0, 2)).astype(bf)
        in_maps.append({
            "hs": hs_packed[b],
            "wq": pack_w(wq[512 * g:512 * (g + 1), :]),
            "wk": pack_w(wk[256 * g:256 * (g + 1), :]),
            "wv": pack_w(wv[256 * g:256 * (g + 1), :]),
            "wo": wo_r,
            "tabs": tabs, "rots": rots, "msk": msk,
        })
    return in_maps


def _postprocess(results):
    out = np.empty((B, S, H), np.float32)
    for b in range(B):
        acc = np.zeros((H, S), np.float32)
        for g in range(4):
            y_r = results[4 * b + g]["y"].astype(np.float32)  # [128, 16, S]
            acc += y_r.transpose(1, 0, 2).reshape(H, S)
        out[b] = acc.T
    return out


def kernel(hidden_states, wq, wk, wv, wo, q_norm_weight, k_norm_weight):
    nc = _build_nc()
    in_maps = _host_inputs(hidden_states, wq, wk, wv, wo,
                           q_norm_weight, k_norm_weight)
    res = run_bass_kernel_spmd(nc, in_maps, list(range(8)))
    return _postprocess(res.results)
